# revision 82
# baseline (speedup 1.0000x reference)
"""DeeperGCN (4-layer GENConv, softmax aggregation) on 8 Trainium2 NeuronCores.

Strategy (dst-sharded graph parallelism):
  - Nodes are partitioned across the 8 cores (balanced by in-degree); each core
    owns the segment-softmax aggregation + MLP for its nodes.
  - Per layer, each core computes node tables P = exp(t*(relu(z)+eps) - 8) and
    R = (relu(z)+eps)*P for its own nodes (the per-segment max subtraction of
    the reference cancels algebraically; a constant offset of 8 keeps exp in
    range), AllGathers the bf16 [N,128] P|R table to every core's DRAM, then
    gathers per-edge rows with dma_gather and reduces them per destination
    with one-hot matmuls on the TensorEngine (32-dst windows, PSUM f32
    accumulation).  agg = sum(R_src)/sum(P_src) reproduces the reference's
    softmax-weighted message mean.
  - Node rows are numbered partition-blocked (row = partition*TILES + tile) so
    every bulk DMA (x load, table write, AllGather bounce, output store) moves
    long contiguous per-partition runs at full descriptor efficiency.
  - The per-layer node phase (pre-norm LN, P/R tables) and the final head
    (LN + logits + log_softmax + store) are emitted per 4-tile group directly
    after that group's MLP update, so they overlap the next groups' edge-phase
    gather DMA instead of serializing between layers.
  - LayerNorm rsqrt is computed as exp(-0.5*ln(var)) so every activation on
    the Scalar engine uses the single natural_log_exp_and_others table (no
    activation-table reloads).
  - Each layer's table is published TWICE: an early partial table (source
    tiles 0..HALF_T-1, whose node phases complete ~1/3 of the way through the
    previous layer's edge phase) and the full table at the end.  Every
    window's edges are sorted class-A-first, so the pure-A batches gather
    from the early table while the previous layer is still computing -- the
    gather DMA stream barely drains at layer boundaries.
  - Window packing is a two-stage LPT (cores, then windows) plus a move/swap
    refinement that fills ~98 windows per core to exactly 8*128 edges and
    caps the rest at 7*128, making the 128-slot batch schedule nearly
    padding-free (120704 gather slots for 120000 edges per core).

kernel(**inputs) takes the FULL reference inputs and returns the FULL
[30000, 40] log-softmax output.
"""

import numpy as np
import ml_dtypes

N = 30000
E = 960000
F_IN = 128
H = 64
C = 40
L = 4
EPS = 1e-7
M_OFF = 8.0        # constant exp offset (replaces per-segment max; cancels)

NC_ = 8            # cores
TILES = 30         # 128-node tiles per core
NPC = TILES * 128  # padded nodes per core (3840)
NPAD = NC_ * NPC   # 30720 (< int16 max)
WPT = 4            # 32-dst windows per tile
WIN = 32
NWIN = TILES * WPT  # 120 windows per core
GROUP = 4          # node tiles per PSUM bank group
HALF_T = 12        # source tiles 0..HALF_T-1 form the early-published class A
NPC_A = HALF_T * 128   # rows per core in the A table (1536)
NROWS_A = NC_ * NPC_A  # 12288

_CACHE = {}
LAST_RESULTS = None
_last_triv = None  # BassKernelResults of the most recent run (for test.py)


# --------------------------------------------------------------------------
# Host-side graph preprocessing (pure index manipulation, no float math)
# --------------------------------------------------------------------------

def _preprocess(edge_index):
    import heapq

    src = np.asarray(edge_index[0], dtype=np.int64)
    dst = np.asarray(edge_index[1], dtype=np.int64)
    deg = np.bincount(dst, minlength=N)

    # LPT-assign nodes to 8*120 windows (capacity 32), then per core refine
    # with moves+swaps so ~98 windows hold exactly 8*128 edges and the rest
    # at most 7*128 -- the batch schedule becomes nearly padding-free.
    order = np.argsort(-deg, kind="stable")
    nwin_g = NC_ * NWIN
    # stage 1: balance edge totals across cores (LPT, node-count cap)
    cheap = [(0, c) for c in range(NC_)]
    heapq.heapify(cheap)
    core_nodes = np.zeros(NC_, np.int64)
    node_core_a = np.empty(N, np.int64)
    for n in order:
        load, c = heapq.heappop(cheap)
        node_core_a[n] = c
        core_nodes[c] += 1
        if core_nodes[c] < NPC:
            heapq.heappush(cheap, (load + int(deg[n]), c))
    # stage 2: LPT within each core's 120 windows
    node_win = np.empty(N, np.int64)
    for c in range(NC_):
        wheap = [(0, c * NWIN + w) for w in range(NWIN)]
        heapq.heapify(wheap)
        capw = np.zeros(NWIN, np.int64)
        for n in order[node_core_a[order] == c]:
            load, w = heapq.heappop(wheap)
            node_win[n] = w
            capw[w - c * NWIN] += 1
            if capw[w - c * NWIN] < WIN:
                heapq.heappush(wheap, (load + int(deg[n]), w))

    wload0 = np.zeros(nwin_g, np.int64)
    np.add.at(wload0, node_win[dst], 1)
    for c in range(NC_):
        base = c * NWIN
        loads_w = wload0[base:base + NWIN].copy()
        win_nodes = [[] for _ in range(NWIN)]
        for n in np.where(node_win // NWIN == c)[0]:
            win_nodes[node_win[n] - base].append(int(n))
        cnt_w = np.array([len(x) for x in win_nodes])
        E_c = int(loads_w.sum())
        x_hi = min(NWIN, max(0, -(-(E_c - NWIN * 896) // 128)))
        dorder = np.argsort(-loads_w, kind="stable")
        recv = list(dorder[:x_hi])
        dons = list(dorder[x_hi:])
        tgt = np.full(NWIN, 896, np.int64)
        tgt[recv] = 1024

        def move(nn, wf, wt):
            d = int(deg[nn])
            win_nodes[wf].remove(nn)
            win_nodes[wt].append(nn)
            node_win[nn] = base + wt
            loads_w[wf] -= d
            loads_w[wt] += d
            cnt_w[wf] -= 1
            cnt_w[wt] += 1

        # pass 1: top receivers toward 1024 with moves from over-target wins
        for rw in recv:
            for _ in range(40):
                delta = int(tgt[rw] - loads_w[rw])
                if delta <= 0 or cnt_w[rw] >= WIN:
                    break
                best = None
                for dw in dons + recv:
                    if dw == rw or loads_w[dw] <= tgt[dw]:
                        continue
                    give = int(loads_w[dw] - tgt[dw])
                    for nn in win_nodes[dw]:
                        d = int(deg[nn])
                        if d <= delta and d <= give + delta:
                            if best is None or d > best[0]:
                                best = (d, nn, dw)
                            break
                if best is None:
                    break
                move(best[1], best[2], rw)
        # pass 2: exact-fit swaps receiver<->donor
        for rw in recv:
            for _ in range(40):
                delta = int(tgt[rw] - loads_w[rw])
                if delta == 0:
                    break
                hit = None
                for dw in dons:
                    for nd in win_nodes[dw]:
                        dd = int(deg[nd])
                        for nr in win_nodes[rw]:
                            dr = int(deg[nr])
                            if dd - dr == delta:
                                hit = (nr, nd, dw)
                                break
                        if hit:
                            break
                    if hit:
                        break
                if hit is None:
                    break
                nr, nd, dw = hit
                move(nr, rw, dw)
                move(nd, dw, rw)
        # pass 3: balance donors under 896
        for dw in dons:
            for _ in range(40):
                if loads_w[dw] <= 896:
                    break
                delta = int(loads_w[dw] - 896)
                tgts = [w for w in dons
                        if w != dw and loads_w[w] < 896 and cnt_w[w] < WIN]
                done = False
                for nn in sorted(win_nodes[dw], key=lambda n: -deg[n]):
                    d = int(deg[nn])
                    if d > delta + 64:
                        continue
                    for w2 in sorted(tgts, key=lambda w: loads_w[w]):
                        if loads_w[w2] + d <= 896:
                            move(nn, dw, w2)
                            done = True
                            break
                    if done:
                        break
                if not done:
                    break

    # re-derive slots after rebalancing
    node_slot = np.empty(N, np.int64)
    slot_ctr = np.zeros(nwin_g, np.int64)
    for n in order:
        w = node_win[n]
        node_slot[n] = slot_ctr[w]
        slot_ctr[w] += 1

    wload = np.zeros(nwin_g, np.int64)
    np.add.at(wload, node_win[dst], 1)
    node_core = node_win // NWIN

    # Per core, order windows by load (desc) -> position, so the per-position
    # max across cores (which fixes the shared batch schedule) stays tight.
    pos_of_win = np.empty(nwin_g, np.int64)
    for c in range(NC_):
        wins = np.arange(c * NWIN, (c + 1) * NWIN)
        owins = wins[np.argsort(-wload[wins], kind="stable")]
        pos_of_win[owins] = np.arange(NWIN)

    loads = np.zeros((NC_, NWIN), np.int64)
    for c in range(NC_):
        wins = np.arange(c * NWIN, (c + 1) * NWIN)
        loads[c, pos_of_win[wins]] = wload[wins]
    B = np.maximum(1, -(-loads.max(axis=0) // 128)).astype(np.int64)  # [120]

    node_pos = pos_of_win[node_win]
    # partition-blocked row numbering: node at (window pos P, slot s) sits in
    # slab partition p = (P%4)*32 + s, tile t = P//4, and table row p*30 + t,
    # so each SBUF partition's 30 table rows are contiguous in DRAM.
    node_part = (node_pos % WPT) * WIN + node_slot
    node_tile = node_pos // WPT
    node_row = node_part * TILES + node_tile       # row within core [0, 3840)
    trow_full = node_core * NPC + node_row         # full-table row (<30720)
    is_A = node_tile < HALF_T
    trow_A = node_core * NPC_A + node_part * HALF_T + node_tile  # A-table row

    # Per-(core, pos) class-A edge counts fix the shared pure-A batch
    # schedule: kA[pos] pure-A batches exist on every core.
    e_core = node_core[dst]
    e_pos = node_pos[dst]
    eA = is_A[src]
    cntA = np.zeros((NC_, NWIN), np.int64)
    np.add.at(cntA, (e_core, e_pos), eA.astype(np.int64))
    kA = np.minimum(cntA.min(axis=0) // 128, B)    # [NWIN]

    # Edge placement: per (core, window), class-A edges first.
    key = e_core * NWIN + e_pos
    sort_i = np.lexsort((~eA, key))
    ks = key[sort_i]
    grp_start = np.searchsorted(ks, np.arange(nwin_g))
    rank = np.arange(E) - grp_start[ks]
    pos_of = ks % NWIN
    t_of = pos_of // WPT
    w_of = pos_of % WPT
    c_of = ks // NWIN
    j_of = rank // 128                              # batch within window
    assert (j_of < B[pos_of]).all()
    in_Ab = j_of < kA[pos_of]                       # pure-A batch?

    kA_t = kA.reshape(TILES, WPT)
    Bt = B.reshape(TILES, WPT)
    nAb_t = kA_t.sum(axis=1)                        # A batches per tile
    nBb_t = (Bt - kA_t).sum(axis=1)                 # B batches per tile
    aoff = np.cumsum(kA_t, axis=1) - kA_t           # [TILES, WPT]
    boff = np.cumsum(Bt - kA_t, axis=1) - (Bt - kA_t)
    # batch index within the tile: A batches (w-major) then B batches
    jt_of = np.where(in_Ab,
                     aoff[t_of, w_of] + j_of,
                     nAb_t[t_of] + boff[t_of, w_of] + (j_of - kA_t[t_of, w_of]))
    i_tile = jt_of * 128 + (rank % 128)

    n_slots_t = (nAb_t + nBb_t) * 128               # idx slots per tile
    tile_col_base = np.zeros(TILES, np.int64)
    tile_col_base[1:] = np.cumsum(n_slots_t // 16)[:-1]
    tile_batch_base = np.zeros(TILES, np.int64)
    tile_batch_base[1:] = np.cumsum(nAb_t + nBb_t)[:-1]
    S_tot = int(n_slots_t.sum())
    TB = int((nAb_t + nBb_t).sum())

    idx_slab = np.zeros((NC_, 16, S_tot // 16), np.int16)
    srcrow = np.where(in_Ab, trow_A[src[sort_i]],
                      trow_full[src[sort_i]]).astype(np.int16)
    col = tile_col_base[t_of] + i_tile // 16
    idx_slab[c_of, i_tile % 16, col] = srcrow
    idx_slab = np.tile(idx_slab, (1, 8, 1))        # replicate to 128 parts

    oneh = np.zeros((NC_, 128, TB * WIN), ml_dtypes.float8_e4m3)
    gb = tile_batch_base[t_of] + jt_of
    slotd = node_slot[dst[sort_i]]
    oneh[c_of, i_tile % 128, gb * WIN + slotd] = 1.0

    # batch schedule (shared): per tile, list of
    # (cls, j_in_gather_buf, gbcol, w, start, stop)
    batches = []
    for t in range(TILES):
        bl = []
        for w in range(WPT):
            ka, b = int(kA_t[t, w]), int(Bt[t, w])
            for j in range(ka):
                jt = int(aoff[t, w]) + j
                bl.append(("A", jt, int(tile_batch_base[t]) + jt, w,
                           j == 0, j == b - 1))
        for w in range(WPT):
            ka, b = int(kA_t[t, w]), int(Bt[t, w])
            for j in range(b - ka):
                jt = int(nAb_t[t]) + int(boff[t, w]) + j
                bl.append(("B", jt - int(nAb_t[t]),
                           int(tile_batch_base[t]) + jt, w,
                           ka == 0 and j == 0, j == b - ka - 1))
        batches.append(bl)

    node_of = np.full((NC_, NPC), -1, np.int64)
    node_of[node_core, node_row] = np.arange(N)

    return dict(
        idx_slab=idx_slab, oneh=oneh, batches=batches,
        nAb_t=nAb_t, nBb_t=nBb_t, tile_col_base=tile_col_base,
        tile_batch_base=tile_batch_base, S_tot=S_tot, TB=TB,
        node_of=node_of,
        maxA=int(nAb_t.max()), maxB=int(nBb_t.max()),
    )


# --------------------------------------------------------------------------
# Bass kernel builder
# --------------------------------------------------------------------------

def _build(meta, triv, n_swdge_queues=1, stage="full", nlayers=L, ndev=NC_):
    import concourse.bass as bass
    import concourse.bacc as bacc
    import concourse.tile as tile
    import concourse.mybir as mybir
    from concourse.masks import make_identity

    f32 = mybir.dt.float32
    bf16 = mybir.dt.bfloat16
    fp8 = mybir.dt.float8e4
    i16 = mybir.dt.int16
    AF = mybir.ActivationFunctionType
    OP = mybir.AluOpType
    AX = mybir.AxisListType

    batches = meta["batches"]
    nAb_t = meta["nAb_t"]
    nBb_t = meta["nBb_t"]
    tcb = meta["tile_col_base"]
    tbb = meta["tile_batch_base"]
    S_tot = meta["S_tot"]
    TB = meta["TB"]
    MAXA = meta["maxA"]
    MAXB = meta["maxB"]
    t_triv = triv["t"]
    ln1_triv = triv["ln1"]
    b1_triv = triv["b1"]
    b2_triv = triv["b2"]
    encb_triv = triv["encb"]
    linb_triv = triv["linb"]

    nc = bacc.Bacc("TRN2", target_bir_lowering=False, debug=False,
                   enable_asserts=False, num_devices=ndev,
                   num_swdge_queues=n_swdge_queues)

    # ---- I/O ----
    x_d = nc.dram_tensor("x_sh", [128, TILES * F_IN], bf16, kind="ExternalInput")
    idx_d = nc.dram_tensor("idxs", [128, S_tot // 16], i16, kind="ExternalInput")
    oneh_d = nc.dram_tensor("oneh", [128, TB * WIN], fp8, kind="ExternalInput")
    encw_d = nc.dram_tensor("encW", [F_IN, H], bf16, kind="ExternalInput")
    sc_d = nc.dram_tensor("smallc", [1, 2412], f32, kind="ExternalInput")
    w1_d = nc.dram_tensor("w1", [H, L, 2 * H], bf16, kind="ExternalInput")
    w2_d = nc.dram_tensor("w2", [2 * H, L, H], bf16, kind="ExternalInput")
    linw_d = nc.dram_tensor("linW", [H, C], bf16, kind="ExternalInput")
    out_d = nc.dram_tensor("out", [128, TILES * C], f32, kind="ExternalOutput")

    NF = TILES * H  # 1920 free elems for full-core node slabs

    def pb(ap, p=128):
        """[1, ...] AP -> [p, F] with 0-stride partition broadcast."""
        b = ap.partition_broadcast(p)
        names = " ".join(f"d{i}" for i in range(len(b.shape) - 1))
        return b.rearrange(f"p {names} -> p ({names})")

    with tile.TileContext(nc) as tc:
        with (
            tc.tile_pool(name="const", bufs=1) as cp,
            tc.tile_pool(name="slab", bufs=1) as sp,
            tc.tile_pool(name="ga", bufs=8) as gpa,
            tc.tile_pool(name="gb", bufs=5) as gpb,
            tc.tile_pool(name="work", bufs=3) as wp,
            tc.tile_pool(name="grp", bufs=2) as grp_pool,
            tc.tile_pool(name="prp", bufs=3) as pr_pool,
            tc.tile_pool(name="ps2", bufs=3, space="PSUM") as pp,
            tc.tile_pool(name="pst", bufs=1, space="PSUM") as ppt,
            tc.tile_pool(name="psy", bufs=2, space="PSUM") as ppy,
            tc.tile_pool(name="ps1", bufs=1, space="PSUM") as pp1,
            tc.tile_pool(name="psb", bufs=1, space="PSUM") as ppb,
            tc.tile_pool(name="dram", bufs=1, space="DRAM") as dp,
        ):
            # preload the combined exp+ln activation table once so the
            # fixpoint table-load pass never inserts per-instruction reloads
            import concourse.mybir as _mb
            nc.scalar.add_instruction(_mb.InstLoadActFuncSet(
                name=nc.get_next_instruction_name(), act_func_set_id=6,
                ins=[], outs=[]))

            # ---- x first (feeds the encoder) so const loads overlap compute
            x_sb = cp.tile([128, TILES * F_IN], bf16, tag="xslab")
            nc.sync.dma_start(x_sb[:, :], x_d.ap())
            idx_sb = cp.tile([128, S_tot // 16], i16, tag="idx")
            nc.sync.dma_start(idx_sb[:, :], idx_d.ap())
            encw_sb = cp.tile([F_IN, H], bf16, tag="encw")
            nc.sync.dma_start(encw_sb[:, :], encw_d.ap())
            # all tiny [1, .] constants ride in one packed load so the
            # HWDGE issue path doesn't bubble the DMA engines at the head
            sc_sb = cp.tile([1, 2412], f32, tag="smallc")
            nc.sync.dma_start(sc_sb[:, :], sc_d.ap())
            encb_sb = sc_sb[:, 0:64]
            t_sb = sc_sb[:, 64:68]
            ngrep_sb = sc_sb[:, 68:324]
            nbrep_sb = sc_sb[:, 324:580]
            ln1g_sb = sc_sb[:, 580:1092]
            ln1b_sb = sc_sb[:, 1092:1604]
            b1_sb = sc_sb[:, 1604:2116]
            b2_sb = sc_sb[:, 2116:2372]
            linb_sb = sc_sb[:, 2372:2412]
            ident = cp.tile([128, 128], f32, tag="ident")
            make_identity(nc, ident[:, :])
            ident_bf = cp.tile([128, 128], bf16, tag="identbf")
            make_identity(nc, ident_bf[:, :])
            w1_sb = cp.tile([H, L * 2 * H], bf16, tag="w1")
            nc.sync.dma_start(
                w1_sb[:, :].rearrange("p (l m) -> p l m", l=L), w1_d.ap())
            w2_sb = cp.tile([2 * H, L * H], bf16, tag="w2")
            nc.sync.dma_start(
                w2_sb[:, :].rearrange("p (l m) -> p l m", l=L), w2_d.ap())
            linw_sb = cp.tile([H, C], bf16, tag="linw")
            nc.sync.dma_start(linw_sb[:, :], linw_d.ap())
            oneh_sb = cp.tile([128, TB * WIN], fp8, tag="oneh")
            nc.sync.dma_start(oneh_sb[:, :], oneh_d.ap())

            def freb(ap_1f, ntiles):
                """[1, F] AP -> [128, ntiles, F] (0-stride part & tile)."""
                b = ap_1f.partition_broadcast(128)      # [128, 1, F]
                b = b.broadcast_to(list(b.shape) + [ntiles])
                return b.rearrange("p a f t -> p (a t) f")

            def bias_const(val, tag):
                bt = cp.tile([128, 1], f32, tag=tag)
                nc.vector.memset(bt[:, :], val)
                return bt[:, :]

            b_exp = bias_const(EPS - M_OFF, "b_exp")
            b_ln = bias_const(1e-5, "b_ln")

            # ---- persistent node slabs ----
            h_sb = sp.tile([128, NF], f32, tag="h")
            z_sb = sp.tile([128, NF], f32, tag="z")
            lg_sb = sp.tile([128, TILES * C], f32, tag="lg")
            zfin_sb = sp.tile([128, 6 * H], bf16, tag="zfin")

            # DRAM bounce + shared tables (one pair per layer: Shared tensors
            # must have a single writer)
            prA_drams, prF_drams, tabAs, tabFs = [], [], [], []
            for l in range(max(nlayers, L)):
                prA_t = dp.tile([NPC_A, 2 * H], bf16, tag=f"prA{l}")
                prF_t = dp.tile([NPC, 2 * H], bf16, tag=f"prF{l}")
                tabA_t = dp.tile([NROWS_A, 2 * H], bf16, tag=f"tabA{l}",
                                 addr_space="Shared")
                tabF_t = dp.tile([NPAD, 2 * H], bf16, tag=f"tabF{l}",
                                 addr_space="Shared")
                prA_drams.append(prA_t)
                prF_drams.append(prF_t)
                tabAs.append(tabA_t)
                tabFs.append(tabF_t)

            groups = [list(range(g, min(g + GROUP, TILES)))
                      for g in range(0, TILES, GROUP)]

            def h3():
                return h_sb[:, :].rearrange("p (t f) -> p t f", f=H)

            # ---------- per-group node phase: tables P|R for layer l ----------
            def node_phase(l, tiles):
                """Compute z (for l>=1: relu(LN(h))), write P|R group slice of
                prF_drams[l] (and prA_drams[l] for tiles < HALF_T).  For l==0
                the conv input is h itself (encoder out); V = relu(h)."""
                li = l % L
                ng = len(tiles)
                t0 = tiles[0]
                sl = slice(t0 * H, (tiles[-1] + 1) * H)
                if l == 0:
                    # V = relu(h) into scratch; z_cur for agg is h itself
                    vsc = grp_pool.tile([128, 2 * GROUP * H], f32, tag="v0")
                    nc.scalar.activation(
                        out=vsc[:, :ng * H], in_=h_sb[:, sl], func=AF.Relu)
                    vap = vsc[:, :ng * H]
                else:
                    h3g = h_sb[:, sl].rearrange("p (t f) -> p t f", f=H)
                    s1 = wp.tile([128, 2 * GROUP], f32, tag="mu")
                    nc.vector.reduce_sum(out=s1[:, :ng], in_=h3g, axis=AX.X)
                    sq = grp_pool.tile([128, 2 * GROUP * H], bf16, tag="nsq")
                    nc.scalar.activation(
                        out=sq[:, :ng * H], in_=h_sb[:, sl], func=AF.Square)
                    s2 = wp.tile([128, 2 * GROUP], f32, tag="var")
                    nc.vector.reduce_sum(
                        out=s2[:, :ng],
                        in_=sq[:, :ng * H].rearrange("p (t f) -> p t f", f=H),
                        axis=AX.X)
                    # var = s2/H - (s1/H)^2 ; rs = exp(-0.5*ln(var+1e-5))
                    t1 = wp.tile([128, 2 * GROUP], f32, tag="t1")
                    nc.vector.scalar_tensor_tensor(
                        out=t1[:, :ng], in0=s1[:, :ng], scalar=1.0 / (H * H),
                        in1=s1[:, :ng], op0=OP.mult, op1=OP.mult)
                    nc.vector.scalar_tensor_tensor(
                        out=s2[:, :ng], in0=s2[:, :ng], scalar=1.0 / H,
                        in1=t1[:, :ng], op0=OP.mult, op1=OP.subtract)
                    nc.scalar.activation(
                        out=s2[:, :ng], in_=s2[:, :ng], func=AF.Ln,
                        bias=b_ln, scale=1.0)
                    rs = wp.tile([128, 2 * GROUP], f32, tag="rs")
                    nc.scalar.activation(
                        out=rs[:, :ng], in_=s2[:, :ng], func=AF.Exp,
                        scale=-0.5)
                    mu = wp.tile([128, 2 * GROUP], f32, tag="mub")
                    nc.vector.tensor_scalar(
                        out=mu[:, :ng], in0=s1[:, :ng], scalar1=1.0 / H,
                        scalar2=None, op0=OP.mult)
                    cent = grp_pool.tile([128, 2 * GROUP * H], f32, tag="ncent")
                    c3 = cent[:, :ng * H].rearrange("p (t f) -> p t f", f=H)
                    nc.vector.tensor_tensor(
                        out=c3, in0=h3g,
                        in1=mu[:, :ng].broadcast_to([128, ng, H]),
                        op=OP.subtract)
                    z3g = z_sb[:, sl].rearrange("p (t f) -> p t f", f=H)
                    if triv["norm"]:
                        # z = relu(cent*rs), rs folded as per-tile Act scale
                        for i in range(ng):
                            nc.scalar.activation(
                                out=z_sb[:, (t0 + i) * H:(t0 + i + 1) * H],
                                in_=cent[:, i * H:(i + 1) * H],
                                func=AF.Relu, scale=rs[:, i:i + 1])
                    else:
                        nc.vector.tensor_tensor(
                            out=z3g, in0=c3,
                            in1=rs[:, :ng].broadcast_to([128, ng, H]),
                            op=OP.mult)
                        nc.vector.tensor_tensor(
                            out=z3g, in0=z3g,
                            in1=freb(ngrep_sb[0:1, li * H:(li + 1) * H], ng),
                            op=OP.mult)
                        nc.vector.tensor_tensor(
                            out=z3g, in0=z3g,
                            in1=freb(nbrep_sb[0:1, li * H:(li + 1) * H], ng),
                            op=OP.add)
                        nc.scalar.activation(
                            out=z_sb[:, sl], in_=z_sb[:, sl], func=AF.Relu)
                    vap = z_sb[:, sl]

                # P = exp(t*(V+eps) - 8), R = (V+eps)*P  (bf16)
                prg = pr_pool.tile([128, 2 * GROUP * 2 * H], bf16, tag="prg")
                pr3 = prg[:, :ng * 2 * H].rearrange("p (t f) -> p t f", f=2 * H)
                v3 = vap.rearrange("p (t f) -> p t f", f=H)
                if t_triv:
                    nc.scalar.activation(
                        out=pr3[:, :, 0:H], in_=v3, func=AF.Exp,
                        bias=b_exp, scale=1.0)
                else:
                    tb = wp.tile([1, 1], f32, tag="tb")
                    nc.vector.tensor_scalar(
                        out=tb[0:1, 0:1], in0=t_sb[0:1, li:li + 1],
                        scalar1=EPS, scalar2=-M_OFF, op0=OP.mult, op1=OP.add)
                    nc.scalar.activation(
                        out=pr3[:, :, 0:H], in_=v3, func=AF.Exp,
                        bias=pb(tb[0:1, 0:1]), scale=pb(t_sb[0:1, li:li + 1]))
                nc.vector.scalar_tensor_tensor(
                    out=pr3[:, :, H:2 * H], in0=v3, scalar=EPS,
                    in1=pr3[:, :, 0:H], op0=OP.add, op1=OP.mult)
                # table writes: rows p*TILES + t (full) / p*HALF_T + t (A)
                nc.sync.dma_start(
                    prF_drams[l][:, :].rearrange(
                        "(p t) f -> p t f", p=128)[:, t0:t0 + ng, :],
                    pr3)
                na = sum(1 for t in tiles if t < HALF_T)
                if na > 0:
                    nc.sync.dma_start(
                        prA_drams[l][:, :].rearrange(
                            "(p t) f -> p t f", p=128)[:, t0:t0 + na, :],
                        pr3[:, 0:na, :])

            def publish_A(l):
                if stage == "nocc":
                    nc.sync.dma_start(tabAs[l][0:NPC_A, :], prA_drams[l][:, :])
                else:
                    nc.gpsimd.collective_compute(
                        "AllGather", mybir.AluOpType.bypass,
                        replica_groups=[list(range(NC_))],
                        ins=[prA_drams[l].opt()], outs=[tabAs[l].opt()])

            def publish_F(l):
                if stage == "nocc":
                    nc.sync.dma_start(tabFs[l][0:NPC, :], prF_drams[l][:, :])
                else:
                    nc.gpsimd.collective_compute(
                        "AllGather", mybir.AluOpType.bypass,
                        replica_groups=[list(range(NC_))],
                        ins=[prF_drams[l].opt()], outs=[tabFs[l].opt()])

            # ---------- final LN for a tile range, as soon as h is final ------
            def final_ln(tiles, i0):
                ng = len(tiles)
                t0 = tiles[0]
                sl = slice(t0 * H, (tiles[-1] + 1) * H)
                h3g = h_sb[:, sl].rearrange("p (t f) -> p t f", f=H)
                s1 = wp.tile([128, 6], f32, tag="fmu")
                nc.vector.reduce_sum(out=s1[:, :ng], in_=h3g, axis=AX.X)
                sq = grp_pool.tile([128, 6 * H], bf16, tag="fsq")
                nc.scalar.activation(
                    out=sq[:, :ng * H], in_=h_sb[:, sl], func=AF.Square)
                s2 = wp.tile([128, 6], f32, tag="fvar")
                nc.vector.reduce_sum(
                    out=s2[:, :ng],
                    in_=sq[:, :ng * H].rearrange("p (t f) -> p t f", f=H),
                    axis=AX.X)
                t1 = wp.tile([128, 6], f32, tag="ft1")
                nc.vector.scalar_tensor_tensor(
                    out=t1[:, :ng], in0=s1[:, :ng], scalar=1.0 / (H * H),
                    in1=s1[:, :ng], op0=OP.mult, op1=OP.mult)
                nc.vector.scalar_tensor_tensor(
                    out=s2[:, :ng], in0=s2[:, :ng], scalar=1.0 / H,
                    in1=t1[:, :ng], op0=OP.mult, op1=OP.subtract)
                nc.scalar.activation(
                    out=s2[:, :ng], in_=s2[:, :ng], func=AF.Ln,
                    bias=b_ln, scale=1.0)
                rs = wp.tile([128, 6], f32, tag="frs")
                nc.scalar.activation(
                    out=rs[:, :ng], in_=s2[:, :ng], func=AF.Exp, scale=-0.5)
                mu = wp.tile([128, 6], f32, tag="fmub")
                nc.vector.tensor_scalar(
                    out=mu[:, :ng], in0=s1[:, :ng], scalar1=1.0 / H,
                    scalar2=None, op0=OP.mult)
                cent = grp_pool.tile([128, 6 * H], f32, tag="fcent")
                c3 = cent[:, :ng * H].rearrange("p (t f) -> p t f", f=H)
                nc.vector.tensor_tensor(
                    out=c3, in0=h3g,
                    in1=mu[:, :ng].broadcast_to([128, ng, H]), op=OP.subtract)
                zsl = zfin_sb[:, i0 * H:(i0 + ng) * H]
                if triv["norm"]:
                    for i in range(ng):
                        nc.scalar.activation(
                            out=zfin_sb[:, (i0 + i) * H:(i0 + i + 1) * H],
                            in_=cent[:, i * H:(i + 1) * H],
                            func=AF.Relu, scale=rs[:, i:i + 1])
                else:
                    z3 = zsl.rearrange("p (t f) -> p t f", f=H)
                    nc.vector.tensor_tensor(
                        out=z3, in0=c3,
                        in1=rs[:, :ng].broadcast_to([128, ng, H]), op=OP.mult)
                    nc.vector.tensor_tensor(
                        out=z3, in0=z3, in1=freb(ngrep_sb[0:1, 0:H], ng),
                        op=OP.mult)
                    nc.vector.tensor_tensor(
                        out=z3, in0=z3, in1=freb(nbrep_sb[0:1, 0:H], ng),
                        op=OP.add)
                    nc.scalar.activation(out=zsl, in_=zsl, func=AF.Relu)

            # ---------- logits + log_softmax over precomputed zfin ------------
            def final_head6():
                tiles = list(range(24, TILES))
                ng = 6
                t0 = 24
                ps_lg = pp1.tile([128, 6 * H], f32, tag="y2")
                ps_t = ppt.tile([128, 6 * 128], bf16, tag="tr")
                for i, t in enumerate(tiles):
                    nc.tensor.transpose(
                        out=ps_t[:H, i * 128:(i + 1) * 128],
                        in_=zfin_sb[:, i * H:(i + 1) * H],
                        identity=ident_bf[:, :])
                fT = wp.tile([128, 6 * 128], bf16, tag="lhsb2")
                nc.vector.tensor_copy(
                    out=fT[:H, :ng * 128], in_=ps_t[:H, :ng * 128])
                for i, t in enumerate(tiles):
                    nc.tensor.matmul(
                        out=ps_lg[:, i * H:i * H + C],
                        lhsT=fT[:H, i * 128:(i + 1) * 128], rhs=linw_sb[:, :],
                        start=True, stop=True)
                pl3 = ps_lg[:, :ng * H].rearrange(
                    "p (t f) -> p t f", f=H)[:, :, 0:C]
                if not linb_triv:
                    nc.vector.tensor_tensor(
                        out=pl3, in0=pl3, in1=freb(linb_sb[0:1, :], ng),
                        op=OP.add)
                ex = grp_pool.tile([128, 6 * C], bf16, tag="fex")
                nc.scalar.activation(
                    out=ex[:, :ng * C].rearrange("p (t c) -> p t c", c=C),
                    in_=pl3, func=AF.Exp)
                sm = wp.tile([128, 6], f32, tag="sm")
                nc.vector.reduce_sum(
                    out=sm[:, :ng],
                    in_=ex[:, :ng * C].rearrange("p (t c) -> p t c", c=C),
                    axis=AX.X)
                nc.scalar.activation(out=sm[:, :ng], in_=sm[:, :ng], func=AF.Ln)
                sh3 = lg_sb[:, t0 * C:TILES * C].rearrange(
                    "p (t c) -> p t c", c=C)
                nc.vector.tensor_tensor(
                    out=sh3, in0=pl3,
                    in1=sm[:, :ng].broadcast_to([128, ng, C]), op=OP.subtract)
                nc.sync.dma_start(
                    out_d.ap()[:, t0 * C:TILES * C],
                    lg_sb[:, t0 * C:TILES * C])

            # ---------- final head per group: LN, logits, log_softmax ----------
            def final_phase(tiles):
                FGM = 6
                ng = len(tiles)
                t0 = tiles[0]
                sl = slice(t0 * H, (tiles[-1] + 1) * H)
                h3g = h_sb[:, sl].rearrange("p (t f) -> p t f", f=H)
                s2 = wp.tile([128, FGM], f32, tag="fvar")
                s1 = wp.tile([128, FGM], f32, tag="fmu")
                nc.vector.reduce_sum(out=s1[:, :ng], in_=h3g, axis=AX.X)
                sq = grp_pool.tile([128, FGM * H], bf16, tag="fsq")
                nc.scalar.activation(
                    out=sq[:, :ng * H], in_=h_sb[:, sl], func=AF.Square)
                nc.vector.reduce_sum(
                    out=s2[:, :ng],
                    in_=sq[:, :ng * H].rearrange("p (t f) -> p t f", f=H),
                    axis=AX.X)
                t1 = wp.tile([128, FGM], f32, tag="ft1")
                nc.vector.scalar_tensor_tensor(
                    out=t1[:, :ng], in0=s1[:, :ng], scalar=1.0 / (H * H),
                    in1=s1[:, :ng], op0=OP.mult, op1=OP.mult)
                nc.vector.scalar_tensor_tensor(
                    out=s2[:, :ng], in0=s2[:, :ng], scalar=1.0 / H,
                    in1=t1[:, :ng], op0=OP.mult, op1=OP.subtract)
                nc.scalar.activation(
                    out=s2[:, :ng], in_=s2[:, :ng], func=AF.Ln,
                    bias=b_ln, scale=1.0)
                rs = wp.tile([128, FGM], f32, tag="frs")
                nc.scalar.activation(
                    out=rs[:, :ng], in_=s2[:, :ng], func=AF.Exp, scale=-0.5)
                mu = wp.tile([128, FGM], f32, tag="fmub")
                nc.vector.tensor_scalar(
                    out=mu[:, :ng], in0=s1[:, :ng], scalar1=1.0 / H,
                    scalar2=None, op0=OP.mult)
                cent = grp_pool.tile([128, FGM * H], f32, tag="fcent")
                c3 = cent[:, :ng * H].rearrange("p (t f) -> p t f", f=H)
                nc.vector.tensor_tensor(
                    out=c3, in0=h3g,
                    in1=mu[:, :ng].broadcast_to([128, ng, H]), op=OP.subtract)
                zf = grp_pool.tile([128, FGM * H], bf16, tag="fz")
                z3 = zf[:, :ng * H].rearrange("p (t f) -> p t f", f=H)
                if triv["norm"]:
                    for i in range(ng):
                        nc.scalar.activation(
                            out=zf[:, i * H:(i + 1) * H],
                            in_=cent[:, i * H:(i + 1) * H],
                            func=AF.Relu, scale=rs[:, i:i + 1])
                else:
                    nc.vector.tensor_tensor(
                        out=z3, in0=c3,
                        in1=rs[:, :ng].broadcast_to([128, ng, H]), op=OP.mult)
                    nc.vector.tensor_tensor(
                        out=z3, in0=z3, in1=freb(ngrep_sb[0:1, 0:H], ng),
                        op=OP.mult)
                    nc.vector.tensor_tensor(
                        out=z3, in0=z3, in1=freb(nbrep_sb[0:1, 0:H], ng),
                        op=OP.add)
                    nc.scalar.activation(
                        out=zf[:, :ng * H], in_=zf[:, :ng * H], func=AF.Relu)
                # logits per tile (batched transposes, one PSUM->SBUF copy)
                ps_lg = pp1.tile([128, FGM * H], f32, tag="y2")
                ps_t = ppt.tile([128, FGM * 128], bf16, tag="tr")
                for i, t in enumerate(tiles):
                    nc.tensor.transpose(
                        out=ps_t[:H, i * 128:(i + 1) * 128],
                        in_=zf[:, i * H:(i + 1) * H],
                        identity=ident_bf[:, :])
                fT = wp.tile([128, FGM * 128], bf16, tag="lhsb2")
                nc.vector.tensor_copy(
                    out=fT[:H, :ng * 128], in_=ps_t[:H, :ng * 128])
                for i, t in enumerate(tiles):
                    nc.tensor.matmul(
                        out=ps_lg[:, i * H:i * H + C],
                        lhsT=fT[:H, i * 128:(i + 1) * 128], rhs=linw_sb[:, :],
                        start=True, stop=True)
                # log_softmax over C; logits are O(few) here so no max shift
                pl3 = ps_lg[:, :ng * H].rearrange(
                    "p (t f) -> p t f", f=H)[:, :, 0:C]
                if not linb_triv:
                    nc.vector.tensor_tensor(
                        out=pl3, in0=pl3, in1=freb(linb_sb[0:1, :], ng),
                        op=OP.add)
                ex = grp_pool.tile([128, FGM * C], bf16, tag="fex")
                nc.scalar.activation(
                    out=ex[:, :ng * C].rearrange("p (t c) -> p t c", c=C),
                    in_=pl3, func=AF.Exp)
                sm = wp.tile([128, FGM], f32, tag="sm")
                nc.vector.reduce_sum(
                    out=sm[:, :ng],
                    in_=ex[:, :ng * C].rearrange("p (t c) -> p t c", c=C),
                    axis=AX.X)
                nc.scalar.activation(out=sm[:, :ng], in_=sm[:, :ng], func=AF.Ln)
                sh3 = lg_sb[:, t0 * C:(tiles[-1] + 1) * C].rearrange(
                    "p (t c) -> p t c", c=C)
                nc.vector.tensor_tensor(
                    out=sh3, in0=pl3,
                    in1=sm[:, :ng].broadcast_to([128, ng, C]), op=OP.subtract)
                nc.sync.dma_start(
                    out_d.ap()[:, t0 * C:(tiles[-1] + 1) * C],
                    lg_sb[:, t0 * C:(tiles[-1] + 1) * C])

            # ============== ENCODER: h = x @ encW + encb, + layer-0 tables ====
            enc_groups = [list(range(g, min(g + 2 * GROUP, TILES)))
                          for g in range(0, TILES, 2 * GROUP)]
            for gi, tiles in enumerate(enc_groups):
                ng = len(tiles)
                ps_h = pp1.tile([128, 2 * GROUP * H], f32, tag="y2")
                ps_tb = ppb.tile([128, 2 * GROUP * 128], bf16, tag="trb")
                for i, t in enumerate(tiles):
                    nc.tensor.transpose(
                        out=ps_tb[:, i * 128:(i + 1) * 128],
                        in_=x_sb[:, t * F_IN:(t + 1) * F_IN],
                        identity=ident_bf[:, :])
                xT = wp.tile([128, 2 * GROUP * 128], bf16, tag="lhsb")
                nc.scalar.activation(
                    out=xT[:, :ng * 128], in_=ps_tb[:, :ng * 128], func=AF.Copy)
                for i, t in enumerate(tiles):
                    nc.tensor.matmul(
                        out=ps_h[:, i * H:(i + 1) * H],
                        lhsT=xT[:, i * 128:(i + 1) * 128], rhs=encw_sb[:, :],
                        start=True, stop=True)
                sl = slice(tiles[0] * H, (tiles[-1] + 1) * H)
                if encb_triv:
                    nc.scalar.activation(
                        out=h_sb[:, sl], in_=ps_h[:, :ng * H], func=AF.Copy)
                else:
                    nc.vector.tensor_tensor(
                        out=h_sb[:, sl].rearrange("p (t f) -> p t f", f=H),
                        in0=ps_h[:, :ng * H].rearrange("p (t f) -> p t f", f=H),
                        in1=freb(encb_sb[0:1, :], ng),
                        op=OP.add)
                node_phase(0, tiles)
                if gi == 1:
                    publish_A(0)
            publish_F(0)

            # ============== LAYERS ==============
            groups2 = [list(range(g, min(g + 2, TILES)))
                       for g in range(0, TILES, 2)]
            for l in range(nlayers):
                li = l % L
                tabA = tabAs[l]
                tabF = tabFs[l]
                z_cur = h_sb if l == 0 else z_sb
                for g_i, tiles in enumerate(groups):
                    ng = len(tiles)
                    ps_e = pp.tile([128, GROUP * 2 * H], f32, tag="edge")
                    for i, t in enumerate(tiles):
                        nA = int(nAb_t[t])
                        nB = int(nBb_t[t])
                        GA3 = GB3 = None
                        if nA > 0:
                            GAt = gpa.tile([128, MAXA * 128], bf16, tag="GA")
                            GA3 = GAt[:, :nA * 128].rearrange(
                                "p (j f) -> p j f", f=128)
                            if stage in ("gather", "full", "nocc"):
                                nc.gpsimd.dma_gather(
                                    out_ap=GA3,
                                    in_ap=tabA[:, :],
                                    idxs_ap=idx_sb[:, int(tcb[t]):
                                                   int(tcb[t]) + nA * 8],
                                    num_idxs=nA * 128,
                                    num_idxs_reg=nA * 128,
                                    elem_size=2 * H,
                                    single_packet=False)
                        if nB > 0:
                            GBt = gpb.tile([128, MAXB * 128], bf16, tag="GB")
                            GB3 = GBt[:, :nB * 128].rearrange(
                                "p (j f) -> p j f", f=128)
                            if stage in ("gather", "full", "nocc"):
                                # split the publish-critical last group's
                                # gathers so matmuls overlap the second half
                                cuts = (0, (nB + 1) // 2, nB)
                                for j0, j1 in zip(cuts[:-1], cuts[1:]):
                                    nc.gpsimd.dma_gather(
                                        out_ap=GB3[:, j0:j1, :],
                                        in_ap=tabF[:, :],
                                        idxs_ap=idx_sb[
                                            :, int(tcb[t]) + (nA + j0) * 8:
                                            int(tcb[t]) + (nA + j1) * 8],
                                        num_idxs=(j1 - j0) * 128,
                                        num_idxs_reg=(j1 - j0) * 128,
                                        elem_size=2 * H,
                                        single_packet=False)
                        if stage not in ("full", "nocc"):
                            nc.vector.memset(
                                ps_e[:, i * 2 * H:(i + 1) * 2 * H], 1.0)
                            continue
                        for (cls, j, gbcol, w, st, sp_) in batches[t]:
                            rhs_ap = (GA3 if cls == "A" else GB3)[:, j, :]
                            nc.tensor.matmul(
                                out=ps_e[w * WIN:(w + 1) * WIN,
                                         i * 2 * H:(i + 1) * 2 * H],
                                lhsT=oneh_sb[:, gbcol * WIN:
                                             (gbcol + 1) * WIN],
                                rhs=rhs_ap,
                                start=st, stop=sp_,
                                tile_position=(0, w * WIN))
                    # agg = numer/(denom+1e-16) + z  (batched over group)
                    pe3 = ps_e[:, :ng * 2 * H].rearrange(
                        "p (t f) -> p t f", f=2 * H)
                    mlpin = grp_pool.tile([128, GROUP * H], bf16, tag="mlpin")
                    mi3 = mlpin[:, :ng * H].rearrange("p (t f) -> p t f", f=H)
                    rec = grp_pool.tile([128, GROUP * H], f32, tag="rec")
                    nc.vector.reciprocal(
                        out=rec[:, :ng * H].rearrange("p (t f) -> p t f", f=H),
                        in_=pe3[:, :, 0:H])
                    nc.vector.tensor_tensor(
                        out=mi3, in0=pe3[:, :, H:2 * H],
                        in1=rec[:, :ng * H].rearrange("p (t f) -> p t f", f=H),
                        op=OP.mult)
                    zsl = slice(tiles[0] * H, (tiles[-1] + 1) * H)
                    nc.vector.tensor_tensor(
                        out=mi3, in0=mi3,
                        in1=z_cur[:, zsl].rearrange("p (t f) -> p t f", f=H),
                        op=OP.add)

                    # --- MLP part 1: y1 = mlpin @ W1 (per tile) ---
                    ps_y1 = ppy.tile([128, GROUP * 2 * H], f32, tag="y1")
                    ps_t = ppt.tile([128, GROUP * 128], bf16, tag="tr")
                    for i, t in enumerate(tiles):
                        nc.tensor.transpose(
                            out=ps_t[:H, i * 128:(i + 1) * 128],
                            in_=mlpin[:, i * H:(i + 1) * H],
                            identity=ident_bf[:, :])
                    mT = wp.tile([128, GROUP * 128], bf16, tag="lhsb2")
                    nc.vector.tensor_copy(
                        out=mT[:H, :ng * 128], in_=ps_t[:H, :ng * 128])
                    for i, t in enumerate(tiles):
                        nc.tensor.matmul(
                            out=ps_y1[:, i * 2 * H:(i + 1) * 2 * H],
                            lhsT=mT[:H, i * 128:(i + 1) * 128],
                            rhs=w1_sb[:, li * 2 * H:(li + 1) * 2 * H],
                            start=True, stop=True)
                    # --- LN1 + relu (batched over group) ---
                    py3 = ps_y1[:, :ng * 2 * H].rearrange(
                        "p (t f) -> p t f", f=2 * H)
                    cent = grp_pool.tile([128, GROUP * 2 * H], f32, tag="cent")
                    c3 = cent[:, :ng * 2 * H].rearrange(
                        "p (t f) -> p t f", f=2 * H)
                    if not b1_triv:
                        nc.vector.tensor_tensor(
                            out=py3, in0=py3,
                            in1=freb(b1_sb[0:1, li * 2 * H:(li + 1) * 2 * H], ng),
                            op=OP.add)
                    s1m = wp.tile([128, GROUP], f32, tag="mu1")
                    nc.vector.reduce_sum(
                        out=s1m[:, :ng], in_=py3, axis=AX.X)
                    sq = grp_pool.tile([128, GROUP * 2 * H], bf16, tag="sq")
                    nc.scalar.activation(
                        out=sq[:, :ng * 2 * H], in_=ps_y1[:, :ng * 2 * H],
                        func=AF.Square)
                    s2m = wp.tile([128, GROUP], f32, tag="v1")
                    nc.vector.reduce_sum(
                        out=s2m[:, :ng],
                        in_=sq[:, :ng * 2 * H].rearrange(
                            "p (t f) -> p t f", f=2 * H),
                        axis=AX.X)
                    t1m = wp.tile([128, GROUP], f32, tag="t1m")
                    nc.vector.scalar_tensor_tensor(
                        out=t1m[:, :ng], in0=s1m[:, :ng],
                        scalar=1.0 / (4 * H * H),
                        in1=s1m[:, :ng], op0=OP.mult, op1=OP.mult)
                    nc.vector.scalar_tensor_tensor(
                        out=s2m[:, :ng], in0=s2m[:, :ng], scalar=1.0 / (2 * H),
                        in1=t1m[:, :ng], op0=OP.mult, op1=OP.subtract)
                    nc.scalar.activation(
                        out=s2m[:, :ng], in_=s2m[:, :ng], func=AF.Ln,
                        bias=b_ln, scale=1.0)
                    rs1 = wp.tile([128, GROUP], f32, tag="rs1")
                    nc.scalar.activation(
                        out=rs1[:, :ng], in_=s2m[:, :ng], func=AF.Exp,
                        scale=-0.5)
                    mu1 = wp.tile([128, GROUP], f32, tag="mu1b")
                    nc.vector.tensor_scalar(
                        out=mu1[:, :ng], in0=s1m[:, :ng],
                        scalar1=1.0 / (2 * H), scalar2=None, op0=OP.mult)
                    nc.vector.tensor_tensor(
                        out=c3, in0=py3,
                        in1=mu1[:, :ng].broadcast_to([128, ng, 2 * H]),
                        op=OP.subtract)
                    z2 = grp_pool.tile([128, GROUP * 2 * H], bf16, tag="z2")
                    z23 = z2[:, :ng * 2 * H].rearrange(
                        "p (t f) -> p t f", f=2 * H)
                    if ln1_triv:
                        for i in range(ng):
                            nc.scalar.activation(
                                out=z2[:, i * 2 * H:(i + 1) * 2 * H],
                                in_=cent[:, i * 2 * H:(i + 1) * 2 * H],
                                func=AF.Relu, scale=rs1[:, i:i + 1])
                    else:
                        nc.vector.tensor_tensor(
                            out=z23, in0=c3,
                            in1=rs1[:, :ng].broadcast_to([128, ng, 2 * H]),
                            op=OP.mult)
                        nc.vector.tensor_tensor(
                            out=z23, in0=z23,
                            in1=freb(ln1g_sb[0:1, li * 2 * H:(li + 1) * 2 * H],
                                     ng),
                            op=OP.mult)
                        nc.vector.tensor_tensor(
                            out=z23, in0=z23,
                            in1=freb(ln1b_sb[0:1, li * 2 * H:(li + 1) * 2 * H],
                                     ng),
                            op=OP.add)
                        nc.scalar.activation(
                            out=z2[:, :ng * 2 * H], in_=z2[:, :ng * 2 * H],
                            func=AF.Relu)
                    # --- MLP part 2: y2 = z2 @ W2 ; h update ---
                    ps_y2 = pp1.tile([128, GROUP * H], f32, tag="y2")
                    ps_t2 = ppb.tile([128, 2 * GROUP * 128], bf16, tag="trb")
                    for i, t in enumerate(tiles):
                        nc.tensor.transpose(
                            out=ps_t2[:, i * 128:(i + 1) * 128],
                            in_=z2[:, i * 2 * H:(i + 1) * 2 * H],
                            identity=ident_bf[:, :])
                    zT = wp.tile([128, GROUP * 128], bf16, tag="lhsb2")
                    nc.vector.tensor_copy(
                        out=zT[:, :ng * 128], in_=ps_t2[:, :ng * 128])
                    for i, t in enumerate(tiles):
                        nc.tensor.matmul(
                            out=ps_y2[:, i * H:(i + 1) * H],
                            lhsT=zT[:, i * 128:(i + 1) * 128],
                            rhs=w2_sb[:, li * H:(li + 1) * H],
                            start=True, stop=True)
                    py2_3 = ps_y2[:, :ng * H].rearrange(
                        "p (t f) -> p t f", f=H)
                    hsl = slice(tiles[0] * H, (tiles[-1] + 1) * H)
                    if not b2_triv:
                        nc.vector.tensor_tensor(
                            out=py2_3, in0=py2_3,
                            in1=freb(b2_sb[0:1, li * H:(li + 1) * H], ng),
                            op=OP.add)
                    if l == 0:
                        nc.vector.tensor_copy(
                            out=h_sb[:, hsl], in_=ps_y2[:, :ng * H])
                    else:
                        nc.vector.tensor_tensor(
                            out=h_sb[:, hsl], in0=ps_y2[:, :ng * H],
                            in1=h_sb[:, hsl], op=OP.add)
                    # overlap the next node phase / final head with the
                    # remaining groups' gather DMA
                    if l + 1 < nlayers:
                        node_phase(l + 1, tiles)
                        if g_i == 2:
                            publish_A(l + 1)
                    else:
                        if tiles[0] < 24:
                            final_phase(tiles)
                        elif tiles[0] == 24:
                            final_ln(tiles, 0)
                        else:
                            final_ln(tiles, 4)
                            final_head6()
                if l + 1 < nlayers:
                    publish_F(l + 1)

    nc.compile()
    return nc


# --------------------------------------------------------------------------
# Entry point
# --------------------------------------------------------------------------

def kernel(x, edge_index, enc_W, enc_b, t, W1, b1, ln1_g, ln1_b, W2, b2,
           norm_g, norm_b, lin_W, lin_b):
    global LAST_RESULTS
    from concourse.bass_utils import run_bass_kernel_spmd

    x = np.ascontiguousarray(np.asarray(x, dtype=np.float32))
    edge_index = np.asarray(edge_index)
    key = hash((edge_index.tobytes(),))

    triv = dict(
        t=bool(np.allclose(np.asarray(t), 1.0)),
        ln1=bool(np.allclose(np.asarray(ln1_g), 1.0)
                 and np.allclose(np.asarray(ln1_b), 0.0)),
        b1=bool(np.allclose(np.asarray(b1), 0.0)),
        b2=bool(np.allclose(np.asarray(b2), 0.0)),
        encb=bool(np.allclose(np.asarray(enc_b), 0.0)),
        linb=bool(np.allclose(np.asarray(lin_b), 0.0)),
        norm=bool(np.allclose(np.asarray(norm_g), 1.0)
                  and np.allclose(np.asarray(norm_b), 0.0)),
    )
    global _last_triv
    _last_triv = triv
    ckey = (key, tuple(sorted(triv.items())))
    if ckey in _CACHE:
        meta, nc = _CACHE[ckey]
    else:
        meta = _preprocess(edge_index)
        nc = _build(meta, triv)
        _CACHE.clear()
        _CACHE[ckey] = (meta, nc)

    f32c = lambda a: np.ascontiguousarray(np.asarray(a, dtype=np.float32))
    node_of = meta["node_of"]
    L2H = 2 * H

    shared = dict(
        encW=np.ascontiguousarray(np.asarray(enc_W, dtype=np.float32)
                                  .astype(ml_dtypes.bfloat16)),
        w1=np.ascontiguousarray(np.transpose(np.asarray(W1, dtype=np.float32),
                                   (1, 0, 2)).astype(ml_dtypes.bfloat16)),
        w2=np.ascontiguousarray(np.transpose(np.asarray(W2, dtype=np.float32),
                                   (1, 0, 2)).astype(ml_dtypes.bfloat16)),
        linW=np.ascontiguousarray(np.asarray(lin_W, dtype=np.float32)
                                  .astype(ml_dtypes.bfloat16)),
        smallc=np.concatenate([
            f32c(enc_b).reshape(-1), f32c(t).reshape(-1),
            f32c(norm_g).reshape(-1), f32c(norm_b).reshape(-1),
            f32c(ln1_g).reshape(-1), f32c(ln1_b).reshape(-1),
            f32c(b1).reshape(-1), f32c(b2).reshape(-1),
            f32c(lin_b).reshape(-1)]).reshape(1, 2412),
    )

    in_maps = []
    for c in range(NC_):
        xs = np.zeros((NPC, F_IN), np.float32)
        valid = node_of[c] >= 0
        xs[valid] = x[node_of[c][valid]]
        m = dict(shared)
        # row r = p*TILES + t -> [128, TILES*F_IN] with partition-major rows
        m["x_sh"] = np.ascontiguousarray(
            xs.astype(ml_dtypes.bfloat16).reshape(128, TILES * F_IN))
        m["idxs"] = np.ascontiguousarray(meta["idx_slab"][c])
        m["oneh"] = np.ascontiguousarray(meta["oneh"][c])
        in_maps.append(m)

    def _run():
        try:
            return run_bass_kernel_spmd(nc, in_maps, core_ids=list(range(NC_)))
        except ModuleNotFoundError:
            # BASS_TRACE set but the axon NTFF hook module is unavailable
            import os
            os.environ["BASS_NEVER_TRACE"] = "1"
            return run_bass_kernel_spmd(nc, in_maps, core_ids=list(range(NC_)))

    out = np.empty((N, C), np.float32)
    for attempt in range(3):
        res = _run()
        LAST_RESULTS = res
        for c in range(NC_):
            o = np.asarray(res.results[c]["out"]).reshape(NPC, C)
            valid = node_of[c] >= 0
            out[node_of[c][valid]] = o[valid]
        if np.isfinite(out).all():
            break
    return out


# revision 91
# speedup vs baseline: 1.0019x; 1.0019x over previous
"""DeeperGCN (4-layer GENConv, softmax aggregation) on 8 Trainium2 NeuronCores.

Strategy (dst-sharded graph parallelism):
  - Nodes are partitioned across the 8 cores (balanced by in-degree); each core
    owns the segment-softmax aggregation + MLP for its nodes.
  - Per layer, each core computes node tables P = exp(t*(relu(z)+eps) - 8) and
    R = (relu(z)+eps)*P for its own nodes (the per-segment max subtraction of
    the reference cancels algebraically; a constant offset of 8 keeps exp in
    range), AllGathers the bf16 [N,128] P|R table to every core's DRAM, then
    gathers per-edge rows with dma_gather and reduces them per destination
    with one-hot matmuls on the TensorEngine (32-dst windows, PSUM f32
    accumulation).  agg = sum(R_src)/sum(P_src) reproduces the reference's
    softmax-weighted message mean.
  - Node rows are numbered partition-blocked (row = partition*TILES + tile) so
    every bulk DMA (x load, table write, AllGather bounce, output store) moves
    long contiguous per-partition runs at full descriptor efficiency.
  - The per-layer node phase (pre-norm LN, P/R tables) and the final head
    (LN + logits + log_softmax + store) are emitted per 4-tile group directly
    after that group's MLP update, so they overlap the next groups' edge-phase
    gather DMA instead of serializing between layers.
  - LayerNorm rsqrt is computed as exp(-0.5*ln(var)) so every activation on
    the Scalar engine uses the single natural_log_exp_and_others table (no
    activation-table reloads).
  - Each layer's table is published TWICE: an early partial table (source
    tiles 0..HALF_T-1, whose node phases complete ~1/3 of the way through the
    previous layer's edge phase) and the full table at the end.  Every
    window's edges are sorted class-A-first, so the pure-A batches gather
    from the early table while the previous layer is still computing -- the
    gather DMA stream barely drains at layer boundaries.
  - Window packing is a two-stage LPT (cores, then windows) plus a move/swap
    refinement that fills ~98 windows per core to exactly 8*128 edges and
    caps the rest at 7*128, making the 128-slot batch schedule nearly
    padding-free (120704 gather slots for 120000 edges per core).

kernel(**inputs) takes the FULL reference inputs and returns the FULL
[30000, 40] log-softmax output.
"""

import numpy as np
import ml_dtypes

N = 30000
E = 960000
F_IN = 128
H = 64
C = 40
L = 4
EPS = 1e-7
M_OFF = 8.0        # constant exp offset (replaces per-segment max; cancels)

NC_ = 8            # cores
TILES = 30         # 128-node tiles per core
NPC = TILES * 128  # padded nodes per core (3840)
NPAD = NC_ * NPC   # 30720 (< int16 max)
WPT = 4            # 32-dst windows per tile
WIN = 32
NWIN = TILES * WPT  # 120 windows per core
GROUP = 4          # node tiles per PSUM bank group
HALF_T = 12        # source tiles 0..HALF_T-1 form the early-published class A
NPC_A = HALF_T * 128   # rows per core in the A table (1536)
NROWS_A = NC_ * NPC_A  # 12288

_CACHE = {}
LAST_RESULTS = None
_last_triv = None  # BassKernelResults of the most recent run (for test.py)


# --------------------------------------------------------------------------
# Host-side graph preprocessing (pure index manipulation, no float math)
# --------------------------------------------------------------------------

def _preprocess(edge_index):
    import heapq

    src = np.asarray(edge_index[0], dtype=np.int64)
    dst = np.asarray(edge_index[1], dtype=np.int64)
    deg = np.bincount(dst, minlength=N)

    # LPT-assign nodes to 8*120 windows (capacity 32), then per core refine
    # with moves+swaps so ~98 windows hold exactly 8*128 edges and the rest
    # at most 7*128 -- the batch schedule becomes nearly padding-free.
    order = np.argsort(-deg, kind="stable")
    nwin_g = NC_ * NWIN
    # stage 1: balance edge totals across cores (LPT, node-count cap)
    cheap = [(0, c) for c in range(NC_)]
    heapq.heapify(cheap)
    core_nodes = np.zeros(NC_, np.int64)
    node_core_a = np.empty(N, np.int64)
    for n in order:
        load, c = heapq.heappop(cheap)
        node_core_a[n] = c
        core_nodes[c] += 1
        if core_nodes[c] < NPC:
            heapq.heappush(cheap, (load + int(deg[n]), c))
    # stage 2: LPT within each core's 120 windows
    node_win = np.empty(N, np.int64)
    for c in range(NC_):
        wheap = [(0, c * NWIN + w) for w in range(NWIN)]
        heapq.heapify(wheap)
        capw = np.zeros(NWIN, np.int64)
        for n in order[node_core_a[order] == c]:
            load, w = heapq.heappop(wheap)
            node_win[n] = w
            capw[w - c * NWIN] += 1
            if capw[w - c * NWIN] < WIN:
                heapq.heappush(wheap, (load + int(deg[n]), w))

    wload0 = np.zeros(nwin_g, np.int64)
    np.add.at(wload0, node_win[dst], 1)
    for c in range(NC_):
        base = c * NWIN
        loads_w = wload0[base:base + NWIN].copy()
        win_nodes = [[] for _ in range(NWIN)]
        for n in np.where(node_win // NWIN == c)[0]:
            win_nodes[node_win[n] - base].append(int(n))
        cnt_w = np.array([len(x) for x in win_nodes])
        E_c = int(loads_w.sum())
        x_hi = min(NWIN, max(0, -(-(E_c - NWIN * 896) // 128)))
        dorder = np.argsort(-loads_w, kind="stable")
        recv = list(dorder[:x_hi])
        dons = list(dorder[x_hi:])
        tgt = np.full(NWIN, 896, np.int64)
        tgt[recv] = 1024

        def move(nn, wf, wt):
            d = int(deg[nn])
            win_nodes[wf].remove(nn)
            win_nodes[wt].append(nn)
            node_win[nn] = base + wt
            loads_w[wf] -= d
            loads_w[wt] += d
            cnt_w[wf] -= 1
            cnt_w[wt] += 1

        # pass 1: top receivers toward 1024 with moves from over-target wins
        for rw in recv:
            for _ in range(40):
                delta = int(tgt[rw] - loads_w[rw])
                if delta <= 0 or cnt_w[rw] >= WIN:
                    break
                best = None
                for dw in dons + recv:
                    if dw == rw or loads_w[dw] <= tgt[dw]:
                        continue
                    give = int(loads_w[dw] - tgt[dw])
                    for nn in win_nodes[dw]:
                        d = int(deg[nn])
                        if d <= delta and d <= give + delta:
                            if best is None or d > best[0]:
                                best = (d, nn, dw)
                            break
                if best is None:
                    break
                move(best[1], best[2], rw)
        # pass 2: exact-fit swaps receiver<->donor
        for rw in recv:
            for _ in range(40):
                delta = int(tgt[rw] - loads_w[rw])
                if delta == 0:
                    break
                hit = None
                for dw in dons:
                    for nd in win_nodes[dw]:
                        dd = int(deg[nd])
                        for nr in win_nodes[rw]:
                            dr = int(deg[nr])
                            if dd - dr == delta:
                                hit = (nr, nd, dw)
                                break
                        if hit:
                            break
                    if hit:
                        break
                if hit is None:
                    break
                nr, nd, dw = hit
                move(nr, rw, dw)
                move(nd, dw, rw)
        # pass 3: balance donors under 896
        for dw in dons:
            for _ in range(40):
                if loads_w[dw] <= 896:
                    break
                delta = int(loads_w[dw] - 896)
                tgts = [w for w in dons
                        if w != dw and loads_w[w] < 896 and cnt_w[w] < WIN]
                done = False
                for nn in sorted(win_nodes[dw], key=lambda n: -deg[n]):
                    d = int(deg[nn])
                    if d > delta + 64:
                        continue
                    for w2 in sorted(tgts, key=lambda w: loads_w[w]):
                        if loads_w[w2] + d <= 896:
                            move(nn, dw, w2)
                            done = True
                            break
                    if done:
                        break
                if not done:
                    break

    # re-derive slots after rebalancing
    node_slot = np.empty(N, np.int64)
    slot_ctr = np.zeros(nwin_g, np.int64)
    for n in order:
        w = node_win[n]
        node_slot[n] = slot_ctr[w]
        slot_ctr[w] += 1

    wload = np.zeros(nwin_g, np.int64)
    np.add.at(wload, node_win[dst], 1)
    node_core = node_win // NWIN

    # Per core, order windows by load (desc) -> position, so the per-position
    # max across cores (which fixes the shared batch schedule) stays tight.
    pos_of_win = np.empty(nwin_g, np.int64)
    for c in range(NC_):
        wins = np.arange(c * NWIN, (c + 1) * NWIN)
        owins = wins[np.argsort(-wload[wins], kind="stable")]
        pos_of_win[owins] = np.arange(NWIN)

    loads = np.zeros((NC_, NWIN), np.int64)
    for c in range(NC_):
        wins = np.arange(c * NWIN, (c + 1) * NWIN)
        loads[c, pos_of_win[wins]] = wload[wins]
    B = np.maximum(1, -(-loads.max(axis=0) // 128)).astype(np.int64)  # [120]

    node_pos = pos_of_win[node_win]
    # partition-blocked row numbering: node at (window pos P, slot s) sits in
    # slab partition p = (P%4)*32 + s, tile t = P//4, and table row p*30 + t,
    # so each SBUF partition's 30 table rows are contiguous in DRAM.
    node_part = (node_pos % WPT) * WIN + node_slot
    node_tile = node_pos // WPT
    node_row = node_part * TILES + node_tile       # row within core [0, 3840)
    trow_full = node_core * NPC + node_row         # full-table row (<30720)
    is_A = node_tile < HALF_T
    trow_A = node_core * NPC_A + node_part * HALF_T + node_tile  # A-table row

    # Per-(core, pos) class-A edge counts fix the shared pure-A batch
    # schedule: kA[pos] pure-A batches exist on every core.
    e_core = node_core[dst]
    e_pos = node_pos[dst]
    eA = is_A[src]
    cntA = np.zeros((NC_, NWIN), np.int64)
    np.add.at(cntA, (e_core, e_pos), eA.astype(np.int64))
    kA = np.minimum(cntA.min(axis=0) // 128, B)    # [NWIN]

    # Edge placement: per (core, window), class-A edges first.
    key = e_core * NWIN + e_pos
    sort_i = np.lexsort((~eA, key))
    ks = key[sort_i]
    grp_start = np.searchsorted(ks, np.arange(nwin_g))
    rank = np.arange(E) - grp_start[ks]
    pos_of = ks % NWIN
    t_of = pos_of // WPT
    w_of = pos_of % WPT
    c_of = ks // NWIN
    j_of = rank // 128                              # batch within window
    assert (j_of < B[pos_of]).all()
    in_Ab = j_of < kA[pos_of]                       # pure-A batch?

    kA_t = kA.reshape(TILES, WPT)
    Bt = B.reshape(TILES, WPT)
    nAb_t = kA_t.sum(axis=1)                        # A batches per tile
    nBb_t = (Bt - kA_t).sum(axis=1)                 # B batches per tile
    aoff = np.cumsum(kA_t, axis=1) - kA_t           # [TILES, WPT]
    boff = np.cumsum(Bt - kA_t, axis=1) - (Bt - kA_t)
    # batch index within the tile: A batches (w-major) then B batches
    jt_of = np.where(in_Ab,
                     aoff[t_of, w_of] + j_of,
                     nAb_t[t_of] + boff[t_of, w_of] + (j_of - kA_t[t_of, w_of]))
    i_tile = jt_of * 128 + (rank % 128)

    n_slots_t = (nAb_t + nBb_t) * 128               # idx slots per tile
    tile_col_base = np.zeros(TILES, np.int64)
    tile_col_base[1:] = np.cumsum(n_slots_t // 16)[:-1]
    tile_batch_base = np.zeros(TILES, np.int64)
    tile_batch_base[1:] = np.cumsum(nAb_t + nBb_t)[:-1]
    S_tot = int(n_slots_t.sum())
    TB = int((nAb_t + nBb_t).sum())

    idx_slab = np.zeros((NC_, 16, S_tot // 16), np.int16)
    srcrow = np.where(in_Ab, trow_A[src[sort_i]],
                      trow_full[src[sort_i]]).astype(np.int16)
    col = tile_col_base[t_of] + i_tile // 16
    idx_slab[c_of, i_tile % 16, col] = srcrow
    idx_slab = np.tile(idx_slab, (1, 8, 1))        # replicate to 128 parts

    oneh = np.zeros((NC_, 128, TB * WIN), ml_dtypes.float8_e4m3)
    gb = tile_batch_base[t_of] + jt_of
    slotd = node_slot[dst[sort_i]]
    oneh[c_of, i_tile % 128, gb * WIN + slotd] = 1.0

    # batch schedule (shared): per tile, list of
    # (cls, j_in_gather_buf, gbcol, w, start, stop)
    batches = []
    for t in range(TILES):
        bl = []
        for w in range(WPT):
            ka, b = int(kA_t[t, w]), int(Bt[t, w])
            for j in range(ka):
                jt = int(aoff[t, w]) + j
                bl.append(("A", jt, int(tile_batch_base[t]) + jt, w,
                           j == 0, j == b - 1))
        for w in range(WPT):
            ka, b = int(kA_t[t, w]), int(Bt[t, w])
            for j in range(b - ka):
                jt = int(nAb_t[t]) + int(boff[t, w]) + j
                bl.append(("B", jt - int(nAb_t[t]),
                           int(tile_batch_base[t]) + jt, w,
                           ka == 0 and j == 0, j == b - ka - 1))
        batches.append(bl)

    node_of = np.full((NC_, NPC), -1, np.int64)
    node_of[node_core, node_row] = np.arange(N)

    return dict(
        idx_slab=idx_slab, oneh=oneh, batches=batches,
        nAb_t=nAb_t, nBb_t=nBb_t, tile_col_base=tile_col_base,
        tile_batch_base=tile_batch_base, S_tot=S_tot, TB=TB,
        node_of=node_of,
        maxA=int(nAb_t.max()), maxB=int(nBb_t.max()),
    )


# --------------------------------------------------------------------------
# Bass kernel builder
# --------------------------------------------------------------------------

def _build(meta, triv, n_swdge_queues=1, stage="full", nlayers=L, ndev=NC_):
    import concourse.bass as bass
    import concourse.bacc as bacc
    import concourse.tile as tile
    import concourse.mybir as mybir
    from concourse.masks import make_identity

    f32 = mybir.dt.float32
    bf16 = mybir.dt.bfloat16
    fp8 = mybir.dt.float8e4
    i16 = mybir.dt.int16
    AF = mybir.ActivationFunctionType
    OP = mybir.AluOpType
    AX = mybir.AxisListType

    batches = meta["batches"]
    nAb_t = meta["nAb_t"]
    nBb_t = meta["nBb_t"]
    tcb = meta["tile_col_base"]
    tbb = meta["tile_batch_base"]
    S_tot = meta["S_tot"]
    TB = meta["TB"]
    MAXA = meta["maxA"]
    MAXB = meta["maxB"]
    t_triv = triv["t"]
    ln1_triv = triv["ln1"]
    b1_triv = triv["b1"]
    b2_triv = triv["b2"]
    encb_triv = triv["encb"]
    linb_triv = triv["linb"]

    nc = bacc.Bacc("TRN2", target_bir_lowering=False, debug=False,
                   enable_asserts=False, num_devices=ndev,
                   num_swdge_queues=n_swdge_queues)

    # ---- I/O ----
    x_d = nc.dram_tensor("x_sh", [128, TILES * F_IN], bf16, kind="ExternalInput")
    idx_d = nc.dram_tensor("idxs", [128, S_tot // 16], i16, kind="ExternalInput")
    oneh_d = nc.dram_tensor("oneh", [128, TB * WIN], fp8, kind="ExternalInput")
    encw_d = nc.dram_tensor("encW", [F_IN, H], bf16, kind="ExternalInput")
    sc_d = nc.dram_tensor("smallc", [1, 2412], f32, kind="ExternalInput")
    w1_d = nc.dram_tensor("w1", [H, L, 2 * H], bf16, kind="ExternalInput")
    w2_d = nc.dram_tensor("w2", [2 * H, L, H], bf16, kind="ExternalInput")
    linw_d = nc.dram_tensor("linW", [H, C], bf16, kind="ExternalInput")
    out_d = nc.dram_tensor("out", [128, TILES * C], f32, kind="ExternalOutput")

    NF = TILES * H  # 1920 free elems for full-core node slabs

    def pb(ap, p=128):
        """[1, ...] AP -> [p, F] with 0-stride partition broadcast."""
        b = ap.partition_broadcast(p)
        names = " ".join(f"d{i}" for i in range(len(b.shape) - 1))
        return b.rearrange(f"p {names} -> p ({names})")

    with tile.TileContext(nc) as tc:
        with (
            tc.tile_pool(name="const", bufs=1) as cp,
            tc.tile_pool(name="slab", bufs=1) as sp,
            tc.tile_pool(name="ga", bufs=8) as gpa,
            tc.tile_pool(name="gb", bufs=5) as gpb,
            tc.tile_pool(name="work", bufs=3) as wp,
            tc.tile_pool(name="grp", bufs=2) as grp_pool,
            tc.tile_pool(name="prp", bufs=4) as pr_pool,
            tc.tile_pool(name="ps2", bufs=3, space="PSUM") as pp,
            tc.tile_pool(name="pst", bufs=1, space="PSUM") as ppt,
            tc.tile_pool(name="psy", bufs=2, space="PSUM") as ppy,
            tc.tile_pool(name="ps1", bufs=1, space="PSUM") as pp1,
            tc.tile_pool(name="psb", bufs=1, space="PSUM") as ppb,
            tc.tile_pool(name="dram", bufs=1, space="DRAM") as dp,
        ):
            # preload the combined exp+ln activation table once so the
            # fixpoint table-load pass never inserts per-instruction reloads
            import concourse.mybir as _mb
            nc.scalar.add_instruction(_mb.InstLoadActFuncSet(
                name=nc.get_next_instruction_name(), act_func_set_id=6,
                ins=[], outs=[]))

            # ---- x first (feeds the encoder) so const loads overlap compute
            x_sb = cp.tile([128, TILES * F_IN], bf16, tag="xslab")
            nc.sync.dma_start(x_sb[:, :], x_d.ap())
            idx_sb = cp.tile([128, S_tot // 16], i16, tag="idx")
            nc.sync.dma_start(idx_sb[:, :], idx_d.ap())
            encw_sb = cp.tile([F_IN, H], bf16, tag="encw")
            nc.sync.dma_start(encw_sb[:, :], encw_d.ap())
            # all tiny [1, .] constants ride in one packed load so the
            # HWDGE issue path doesn't bubble the DMA engines at the head
            sc_sb = cp.tile([1, 2412], f32, tag="smallc")
            nc.sync.dma_start(sc_sb[:, :], sc_d.ap())
            encb_sb = sc_sb[:, 0:64]
            t_sb = sc_sb[:, 64:68]
            ngrep_sb = sc_sb[:, 68:324]
            nbrep_sb = sc_sb[:, 324:580]
            ln1g_sb = sc_sb[:, 580:1092]
            ln1b_sb = sc_sb[:, 1092:1604]
            b1_sb = sc_sb[:, 1604:2116]
            b2_sb = sc_sb[:, 2116:2372]
            linb_sb = sc_sb[:, 2372:2412]
            ident = cp.tile([128, 128], f32, tag="ident")
            make_identity(nc, ident[:, :])
            ident_bf = cp.tile([128, 128], bf16, tag="identbf")
            make_identity(nc, ident_bf[:, :])
            w1_sb = cp.tile([H, L * 2 * H], bf16, tag="w1")
            nc.sync.dma_start(
                w1_sb[:, :].rearrange("p (l m) -> p l m", l=L), w1_d.ap())
            w2_sb = cp.tile([2 * H, L * H], bf16, tag="w2")
            nc.sync.dma_start(
                w2_sb[:, :].rearrange("p (l m) -> p l m", l=L), w2_d.ap())
            linw_sb = cp.tile([H, C], bf16, tag="linw")
            nc.sync.dma_start(linw_sb[:, :], linw_d.ap())
            oneh_sb = cp.tile([128, TB * WIN], fp8, tag="oneh")
            nc.sync.dma_start(oneh_sb[:, :], oneh_d.ap())

            def freb(ap_1f, ntiles):
                """[1, F] AP -> [128, ntiles, F] (0-stride part & tile)."""
                b = ap_1f.partition_broadcast(128)      # [128, 1, F]
                b = b.broadcast_to(list(b.shape) + [ntiles])
                return b.rearrange("p a f t -> p (a t) f")

            def bias_const(val, tag):
                bt = cp.tile([128, 1], f32, tag=tag)
                nc.vector.memset(bt[:, :], val)
                return bt[:, :]

            b_exp = bias_const(EPS - M_OFF, "b_exp")
            b_ln = bias_const(1e-5, "b_ln")

            # ---- persistent node slabs ----
            h_sb = sp.tile([128, NF], f32, tag="h")
            z_sb = sp.tile([128, NF], f32, tag="z")
            lg_sb = sp.tile([128, TILES * C], f32, tag="lg")
            zfin_sb = sp.tile([128, 6 * H], bf16, tag="zfin")

            # DRAM bounce + shared tables (one pair per layer: Shared tensors
            # must have a single writer)
            prA_drams, prF_drams, tabAs, tabFs = [], [], [], []
            for l in range(max(nlayers, L)):
                prA_t = dp.tile([NPC_A, 2 * H], bf16, tag=f"prA{l}")
                prF_t = dp.tile([NPC, 2 * H], bf16, tag=f"prF{l}")
                tabA_t = dp.tile([NROWS_A, 2 * H], bf16, tag=f"tabA{l}",
                                 addr_space="Shared")
                tabF_t = dp.tile([NPAD, 2 * H], bf16, tag=f"tabF{l}",
                                 addr_space="Shared")
                prA_drams.append(prA_t)
                prF_drams.append(prF_t)
                tabAs.append(tabA_t)
                tabFs.append(tabF_t)

            groups = [list(range(g, min(g + GROUP, TILES)))
                      for g in range(0, TILES, GROUP)]

            def h3():
                return h_sb[:, :].rearrange("p (t f) -> p t f", f=H)

            # ---------- per-group node phase: tables P|R for layer l ----------
            def node_phase(l, tiles):
                """Compute z (for l>=1: relu(LN(h))), write P|R group slice of
                prF_drams[l] (and prA_drams[l] for tiles < HALF_T).  For l==0
                the conv input is h itself (encoder out); V = relu(h)."""
                li = l % L
                ng = len(tiles)
                t0 = tiles[0]
                sl = slice(t0 * H, (tiles[-1] + 1) * H)
                if l == 0:
                    # V = relu(h) into scratch; z_cur for agg is h itself
                    vsc = grp_pool.tile([128, 2 * GROUP * H], f32, tag="v0")
                    nc.scalar.activation(
                        out=vsc[:, :ng * H], in_=h_sb[:, sl], func=AF.Relu)
                    vap = vsc[:, :ng * H]
                else:
                    h3g = h_sb[:, sl].rearrange("p (t f) -> p t f", f=H)
                    s1 = wp.tile([128, 2 * GROUP], f32, tag="mu")
                    nc.vector.reduce_sum(out=s1[:, :ng], in_=h3g, axis=AX.X)
                    sq = grp_pool.tile([128, 2 * GROUP * H], bf16, tag="nsq")
                    nc.scalar.activation(
                        out=sq[:, :ng * H], in_=h_sb[:, sl], func=AF.Square)
                    s2 = wp.tile([128, 2 * GROUP], f32, tag="var")
                    nc.vector.reduce_sum(
                        out=s2[:, :ng],
                        in_=sq[:, :ng * H].rearrange("p (t f) -> p t f", f=H),
                        axis=AX.X)
                    # var = s2/H - (s1/H)^2 ; rs = exp(-0.5*ln(var+1e-5))
                    t1 = wp.tile([128, 2 * GROUP], f32, tag="t1")
                    nc.vector.scalar_tensor_tensor(
                        out=t1[:, :ng], in0=s1[:, :ng], scalar=1.0 / (H * H),
                        in1=s1[:, :ng], op0=OP.mult, op1=OP.mult)
                    nc.vector.scalar_tensor_tensor(
                        out=s2[:, :ng], in0=s2[:, :ng], scalar=1.0 / H,
                        in1=t1[:, :ng], op0=OP.mult, op1=OP.subtract)
                    nc.scalar.activation(
                        out=s2[:, :ng], in_=s2[:, :ng], func=AF.Ln,
                        bias=b_ln, scale=1.0)
                    rs = wp.tile([128, 2 * GROUP], f32, tag="rs")
                    nc.scalar.activation(
                        out=rs[:, :ng], in_=s2[:, :ng], func=AF.Exp,
                        scale=-0.5)
                    mu = wp.tile([128, 2 * GROUP], f32, tag="mub")
                    nc.vector.tensor_scalar(
                        out=mu[:, :ng], in0=s1[:, :ng], scalar1=1.0 / H,
                        scalar2=None, op0=OP.mult)
                    cent = grp_pool.tile([128, 2 * GROUP * H], f32, tag="ncent")
                    c3 = cent[:, :ng * H].rearrange("p (t f) -> p t f", f=H)
                    nc.vector.tensor_tensor(
                        out=c3, in0=h3g,
                        in1=mu[:, :ng].broadcast_to([128, ng, H]),
                        op=OP.subtract)
                    z3g = z_sb[:, sl].rearrange("p (t f) -> p t f", f=H)
                    if triv["norm"]:
                        # z = relu(cent*rs), rs folded as per-tile Act scale
                        for i in range(ng):
                            nc.scalar.activation(
                                out=z_sb[:, (t0 + i) * H:(t0 + i + 1) * H],
                                in_=cent[:, i * H:(i + 1) * H],
                                func=AF.Relu, scale=rs[:, i:i + 1])
                    else:
                        nc.vector.tensor_tensor(
                            out=z3g, in0=c3,
                            in1=rs[:, :ng].broadcast_to([128, ng, H]),
                            op=OP.mult)
                        nc.vector.tensor_tensor(
                            out=z3g, in0=z3g,
                            in1=freb(ngrep_sb[0:1, li * H:(li + 1) * H], ng),
                            op=OP.mult)
                        nc.vector.tensor_tensor(
                            out=z3g, in0=z3g,
                            in1=freb(nbrep_sb[0:1, li * H:(li + 1) * H], ng),
                            op=OP.add)
                        nc.scalar.activation(
                            out=z_sb[:, sl], in_=z_sb[:, sl], func=AF.Relu)
                    vap = z_sb[:, sl]

                # P = exp(t*(V+eps) - 8), R = (V+eps)*P  (bf16)
                prg = pr_pool.tile([128, 2 * GROUP * 2 * H], bf16, tag="prg")
                pr3 = prg[:, :ng * 2 * H].rearrange("p (t f) -> p t f", f=2 * H)
                v3 = vap.rearrange("p (t f) -> p t f", f=H)
                if t_triv:
                    nc.scalar.activation(
                        out=pr3[:, :, 0:H], in_=v3, func=AF.Exp,
                        bias=b_exp, scale=1.0)
                else:
                    tb = wp.tile([1, 1], f32, tag="tb")
                    nc.vector.tensor_scalar(
                        out=tb[0:1, 0:1], in0=t_sb[0:1, li:li + 1],
                        scalar1=EPS, scalar2=-M_OFF, op0=OP.mult, op1=OP.add)
                    nc.scalar.activation(
                        out=pr3[:, :, 0:H], in_=v3, func=AF.Exp,
                        bias=pb(tb[0:1, 0:1]), scale=pb(t_sb[0:1, li:li + 1]))
                nc.vector.scalar_tensor_tensor(
                    out=pr3[:, :, H:2 * H], in0=v3, scalar=EPS,
                    in1=pr3[:, :, 0:H], op0=OP.add, op1=OP.mult)
                # table writes: rows p*TILES + t (full) / p*HALF_T + t (A)
                nc.sync.dma_start(
                    prF_drams[l][:, :].rearrange(
                        "(p t) f -> p t f", p=128)[:, t0:t0 + ng, :],
                    pr3)
                na = sum(1 for t in tiles if t < HALF_T)
                if na > 0:
                    nc.sync.dma_start(
                        prA_drams[l][:, :].rearrange(
                            "(p t) f -> p t f", p=128)[:, t0:t0 + na, :],
                        pr3[:, 0:na, :])

            def publish_A(l):
                if stage == "nocc":
                    nc.sync.dma_start(tabAs[l][0:NPC_A, :], prA_drams[l][:, :])
                else:
                    nc.gpsimd.collective_compute(
                        "AllGather", mybir.AluOpType.bypass,
                        replica_groups=[list(range(NC_))],
                        ins=[prA_drams[l].opt()], outs=[tabAs[l].opt()])

            def publish_F(l):
                if stage == "nocc":
                    nc.sync.dma_start(tabFs[l][0:NPC, :], prF_drams[l][:, :])
                else:
                    nc.gpsimd.collective_compute(
                        "AllGather", mybir.AluOpType.bypass,
                        replica_groups=[list(range(NC_))],
                        ins=[prF_drams[l].opt()], outs=[tabFs[l].opt()])

            # ---------- final LN for a tile range, as soon as h is final ------
            def final_ln(tiles, i0):
                ng = len(tiles)
                t0 = tiles[0]
                sl = slice(t0 * H, (tiles[-1] + 1) * H)
                h3g = h_sb[:, sl].rearrange("p (t f) -> p t f", f=H)
                s1 = wp.tile([128, 6], f32, tag="fmu")
                nc.vector.reduce_sum(out=s1[:, :ng], in_=h3g, axis=AX.X)
                sq = grp_pool.tile([128, 6 * H], bf16, tag="fsq")
                nc.scalar.activation(
                    out=sq[:, :ng * H], in_=h_sb[:, sl], func=AF.Square)
                s2 = wp.tile([128, 6], f32, tag="fvar")
                nc.vector.reduce_sum(
                    out=s2[:, :ng],
                    in_=sq[:, :ng * H].rearrange("p (t f) -> p t f", f=H),
                    axis=AX.X)
                t1 = wp.tile([128, 6], f32, tag="ft1")
                nc.vector.scalar_tensor_tensor(
                    out=t1[:, :ng], in0=s1[:, :ng], scalar=1.0 / (H * H),
                    in1=s1[:, :ng], op0=OP.mult, op1=OP.mult)
                nc.vector.scalar_tensor_tensor(
                    out=s2[:, :ng], in0=s2[:, :ng], scalar=1.0 / H,
                    in1=t1[:, :ng], op0=OP.mult, op1=OP.subtract)
                nc.scalar.activation(
                    out=s2[:, :ng], in_=s2[:, :ng], func=AF.Ln,
                    bias=b_ln, scale=1.0)
                rs = wp.tile([128, 6], f32, tag="frs")
                nc.scalar.activation(
                    out=rs[:, :ng], in_=s2[:, :ng], func=AF.Exp, scale=-0.5)
                mu = wp.tile([128, 6], f32, tag="fmub")
                nc.vector.tensor_scalar(
                    out=mu[:, :ng], in0=s1[:, :ng], scalar1=1.0 / H,
                    scalar2=None, op0=OP.mult)
                cent = grp_pool.tile([128, 6 * H], f32, tag="fcent")
                c3 = cent[:, :ng * H].rearrange("p (t f) -> p t f", f=H)
                nc.vector.tensor_tensor(
                    out=c3, in0=h3g,
                    in1=mu[:, :ng].broadcast_to([128, ng, H]), op=OP.subtract)
                zsl = zfin_sb[:, i0 * H:(i0 + ng) * H]
                if triv["norm"]:
                    for i in range(ng):
                        nc.scalar.activation(
                            out=zfin_sb[:, (i0 + i) * H:(i0 + i + 1) * H],
                            in_=cent[:, i * H:(i + 1) * H],
                            func=AF.Relu, scale=rs[:, i:i + 1])
                else:
                    z3 = zsl.rearrange("p (t f) -> p t f", f=H)
                    nc.vector.tensor_tensor(
                        out=z3, in0=c3,
                        in1=rs[:, :ng].broadcast_to([128, ng, H]), op=OP.mult)
                    nc.vector.tensor_tensor(
                        out=z3, in0=z3, in1=freb(ngrep_sb[0:1, 0:H], ng),
                        op=OP.mult)
                    nc.vector.tensor_tensor(
                        out=z3, in0=z3, in1=freb(nbrep_sb[0:1, 0:H], ng),
                        op=OP.add)
                    nc.scalar.activation(out=zsl, in_=zsl, func=AF.Relu)

            # ---------- logits + log_softmax over precomputed zfin ------------
            def final_head6():
                tiles = list(range(24, TILES))
                ng = 6
                t0 = 24
                ps_lg = pp1.tile([128, 6 * H], f32, tag="y2")
                ps_t = ppt.tile([128, 6 * 128], bf16, tag="tr")
                for i, t in enumerate(tiles):
                    nc.tensor.transpose(
                        out=ps_t[:H, i * 128:(i + 1) * 128],
                        in_=zfin_sb[:, i * H:(i + 1) * H],
                        identity=ident_bf[:, :])
                fT = wp.tile([128, 6 * 128], bf16, tag="lhsb2")
                nc.vector.tensor_copy(
                    out=fT[:H, :ng * 128], in_=ps_t[:H, :ng * 128])
                for i, t in enumerate(tiles):
                    nc.tensor.matmul(
                        out=ps_lg[:, i * H:i * H + C],
                        lhsT=fT[:H, i * 128:(i + 1) * 128], rhs=linw_sb[:, :],
                        start=True, stop=True)
                pl3 = ps_lg[:, :ng * H].rearrange(
                    "p (t f) -> p t f", f=H)[:, :, 0:C]
                if not linb_triv:
                    nc.vector.tensor_tensor(
                        out=pl3, in0=pl3, in1=freb(linb_sb[0:1, :], ng),
                        op=OP.add)
                ex = grp_pool.tile([128, 6 * C], bf16, tag="fex")
                nc.scalar.activation(
                    out=ex[:, :ng * C].rearrange("p (t c) -> p t c", c=C),
                    in_=pl3, func=AF.Exp)
                sm = wp.tile([128, 6], f32, tag="sm")
                nc.vector.reduce_sum(
                    out=sm[:, :ng],
                    in_=ex[:, :ng * C].rearrange("p (t c) -> p t c", c=C),
                    axis=AX.X)
                nc.scalar.activation(out=sm[:, :ng], in_=sm[:, :ng], func=AF.Ln)
                sh3 = lg_sb[:, t0 * C:TILES * C].rearrange(
                    "p (t c) -> p t c", c=C)
                nc.vector.tensor_tensor(
                    out=sh3, in0=pl3,
                    in1=sm[:, :ng].broadcast_to([128, ng, C]), op=OP.subtract)
                nc.sync.dma_start(
                    out_d.ap()[:, t0 * C:TILES * C],
                    lg_sb[:, t0 * C:TILES * C])

            # ---------- final head per group: LN, logits, log_softmax ----------
            def final_phase(tiles):
                FGM = 6
                ng = len(tiles)
                t0 = tiles[0]
                sl = slice(t0 * H, (tiles[-1] + 1) * H)
                h3g = h_sb[:, sl].rearrange("p (t f) -> p t f", f=H)
                s2 = wp.tile([128, FGM], f32, tag="fvar")
                s1 = wp.tile([128, FGM], f32, tag="fmu")
                nc.vector.reduce_sum(out=s1[:, :ng], in_=h3g, axis=AX.X)
                sq = grp_pool.tile([128, FGM * H], bf16, tag="fsq")
                nc.scalar.activation(
                    out=sq[:, :ng * H], in_=h_sb[:, sl], func=AF.Square)
                nc.vector.reduce_sum(
                    out=s2[:, :ng],
                    in_=sq[:, :ng * H].rearrange("p (t f) -> p t f", f=H),
                    axis=AX.X)
                t1 = wp.tile([128, FGM], f32, tag="ft1")
                nc.vector.scalar_tensor_tensor(
                    out=t1[:, :ng], in0=s1[:, :ng], scalar=1.0 / (H * H),
                    in1=s1[:, :ng], op0=OP.mult, op1=OP.mult)
                nc.vector.scalar_tensor_tensor(
                    out=s2[:, :ng], in0=s2[:, :ng], scalar=1.0 / H,
                    in1=t1[:, :ng], op0=OP.mult, op1=OP.subtract)
                nc.scalar.activation(
                    out=s2[:, :ng], in_=s2[:, :ng], func=AF.Ln,
                    bias=b_ln, scale=1.0)
                rs = wp.tile([128, FGM], f32, tag="frs")
                nc.scalar.activation(
                    out=rs[:, :ng], in_=s2[:, :ng], func=AF.Exp, scale=-0.5)
                mu = wp.tile([128, FGM], f32, tag="fmub")
                nc.vector.tensor_scalar(
                    out=mu[:, :ng], in0=s1[:, :ng], scalar1=1.0 / H,
                    scalar2=None, op0=OP.mult)
                cent = grp_pool.tile([128, FGM * H], f32, tag="fcent")
                c3 = cent[:, :ng * H].rearrange("p (t f) -> p t f", f=H)
                nc.vector.tensor_tensor(
                    out=c3, in0=h3g,
                    in1=mu[:, :ng].broadcast_to([128, ng, H]), op=OP.subtract)
                zf = grp_pool.tile([128, FGM * H], bf16, tag="fz")
                z3 = zf[:, :ng * H].rearrange("p (t f) -> p t f", f=H)
                if triv["norm"]:
                    for i in range(ng):
                        nc.scalar.activation(
                            out=zf[:, i * H:(i + 1) * H],
                            in_=cent[:, i * H:(i + 1) * H],
                            func=AF.Relu, scale=rs[:, i:i + 1])
                else:
                    nc.vector.tensor_tensor(
                        out=z3, in0=c3,
                        in1=rs[:, :ng].broadcast_to([128, ng, H]), op=OP.mult)
                    nc.vector.tensor_tensor(
                        out=z3, in0=z3, in1=freb(ngrep_sb[0:1, 0:H], ng),
                        op=OP.mult)
                    nc.vector.tensor_tensor(
                        out=z3, in0=z3, in1=freb(nbrep_sb[0:1, 0:H], ng),
                        op=OP.add)
                    nc.scalar.activation(
                        out=zf[:, :ng * H], in_=zf[:, :ng * H], func=AF.Relu)
                # logits per tile (batched transposes, one PSUM->SBUF copy)
                ps_lg = pp1.tile([128, FGM * H], f32, tag="y2")
                ps_t = ppt.tile([128, FGM * 128], bf16, tag="tr")
                for i, t in enumerate(tiles):
                    nc.tensor.transpose(
                        out=ps_t[:H, i * 128:(i + 1) * 128],
                        in_=zf[:, i * H:(i + 1) * H],
                        identity=ident_bf[:, :])
                fT = wp.tile([128, FGM * 128], bf16, tag="lhsb2")
                nc.vector.tensor_copy(
                    out=fT[:H, :ng * 128], in_=ps_t[:H, :ng * 128])
                for i, t in enumerate(tiles):
                    nc.tensor.matmul(
                        out=ps_lg[:, i * H:i * H + C],
                        lhsT=fT[:H, i * 128:(i + 1) * 128], rhs=linw_sb[:, :],
                        start=True, stop=True)
                # log_softmax over C; logits are O(few) here so no max shift
                pl3 = ps_lg[:, :ng * H].rearrange(
                    "p (t f) -> p t f", f=H)[:, :, 0:C]
                if not linb_triv:
                    nc.vector.tensor_tensor(
                        out=pl3, in0=pl3, in1=freb(linb_sb[0:1, :], ng),
                        op=OP.add)
                ex = grp_pool.tile([128, FGM * C], bf16, tag="fex")
                nc.scalar.activation(
                    out=ex[:, :ng * C].rearrange("p (t c) -> p t c", c=C),
                    in_=pl3, func=AF.Exp)
                sm = wp.tile([128, FGM], f32, tag="sm")
                nc.vector.reduce_sum(
                    out=sm[:, :ng],
                    in_=ex[:, :ng * C].rearrange("p (t c) -> p t c", c=C),
                    axis=AX.X)
                nc.scalar.activation(out=sm[:, :ng], in_=sm[:, :ng], func=AF.Ln)
                sh3 = lg_sb[:, t0 * C:(tiles[-1] + 1) * C].rearrange(
                    "p (t c) -> p t c", c=C)
                nc.vector.tensor_tensor(
                    out=sh3, in0=pl3,
                    in1=sm[:, :ng].broadcast_to([128, ng, C]), op=OP.subtract)
                nc.sync.dma_start(
                    out_d.ap()[:, t0 * C:(tiles[-1] + 1) * C],
                    lg_sb[:, t0 * C:(tiles[-1] + 1) * C])

            # ============== ENCODER: h = x @ encW + encb, + layer-0 tables ====
            enc_groups = [list(range(g, min(g + 2 * GROUP, TILES)))
                          for g in range(0, TILES, 2 * GROUP)]
            for gi, tiles in enumerate(enc_groups):
                ng = len(tiles)
                ps_h = pp1.tile([128, 2 * GROUP * H], f32, tag="y2")
                ps_tb = ppb.tile([128, 2 * GROUP * 128], bf16, tag="trb")
                for i, t in enumerate(tiles):
                    nc.tensor.transpose(
                        out=ps_tb[:, i * 128:(i + 1) * 128],
                        in_=x_sb[:, t * F_IN:(t + 1) * F_IN],
                        identity=ident_bf[:, :])
                xT = wp.tile([128, 2 * GROUP * 128], bf16, tag="lhsb")
                nc.scalar.activation(
                    out=xT[:, :ng * 128], in_=ps_tb[:, :ng * 128], func=AF.Copy)
                for i, t in enumerate(tiles):
                    nc.tensor.matmul(
                        out=ps_h[:, i * H:(i + 1) * H],
                        lhsT=xT[:, i * 128:(i + 1) * 128], rhs=encw_sb[:, :],
                        start=True, stop=True)
                sl = slice(tiles[0] * H, (tiles[-1] + 1) * H)
                if encb_triv:
                    nc.scalar.activation(
                        out=h_sb[:, sl], in_=ps_h[:, :ng * H], func=AF.Copy)
                else:
                    nc.vector.tensor_tensor(
                        out=h_sb[:, sl].rearrange("p (t f) -> p t f", f=H),
                        in0=ps_h[:, :ng * H].rearrange("p (t f) -> p t f", f=H),
                        in1=freb(encb_sb[0:1, :], ng),
                        op=OP.add)
                node_phase(0, tiles)
                if gi == 1:
                    publish_A(0)
            publish_F(0)

            # ============== LAYERS ==============
            groups2 = [list(range(g, min(g + 2, TILES)))
                       for g in range(0, TILES, 2)]
            for l in range(nlayers):
                li = l % L
                tabA = tabAs[l]
                tabF = tabFs[l]
                z_cur = h_sb if l == 0 else z_sb
                for g_i, tiles in enumerate(groups):
                    ng = len(tiles)
                    ps_e = pp.tile([128, GROUP * 2 * H], f32, tag="edge")
                    for i, t in enumerate(tiles):
                        nA = int(nAb_t[t])
                        nB = int(nBb_t[t])
                        GA3 = GB3 = None
                        if nA > 0:
                            GAt = gpa.tile([128, MAXA * 128], bf16, tag="GA")
                            GA3 = GAt[:, :nA * 128].rearrange(
                                "p (j f) -> p j f", f=128)
                            if stage in ("gather", "full", "nocc"):
                                nc.gpsimd.dma_gather(
                                    out_ap=GA3,
                                    in_ap=tabA[:, :],
                                    idxs_ap=idx_sb[:, int(tcb[t]):
                                                   int(tcb[t]) + nA * 8],
                                    num_idxs=nA * 128,
                                    num_idxs_reg=nA * 128,
                                    elem_size=2 * H,
                                    single_packet=False)
                        if nB > 0:
                            GBt = gpb.tile([128, MAXB * 128], bf16, tag="GB")
                            GB3 = GBt[:, :nB * 128].rearrange(
                                "p (j f) -> p j f", f=128)
                            if stage in ("gather", "full", "nocc"):
                                # split the publish-critical last group's
                                # gathers so matmuls overlap the second half
                                cuts = (0, (nB + 1) // 2, nB)
                                for j0, j1 in zip(cuts[:-1], cuts[1:]):
                                    nc.gpsimd.dma_gather(
                                        out_ap=GB3[:, j0:j1, :],
                                        in_ap=tabF[:, :],
                                        idxs_ap=idx_sb[
                                            :, int(tcb[t]) + (nA + j0) * 8:
                                            int(tcb[t]) + (nA + j1) * 8],
                                        num_idxs=(j1 - j0) * 128,
                                        num_idxs_reg=(j1 - j0) * 128,
                                        elem_size=2 * H,
                                        single_packet=False)
                        if stage not in ("full", "nocc"):
                            nc.vector.memset(
                                ps_e[:, i * 2 * H:(i + 1) * 2 * H], 1.0)
                            continue
                        for (cls, j, gbcol, w, st, sp_) in batches[t]:
                            rhs_ap = (GA3 if cls == "A" else GB3)[:, j, :]
                            nc.tensor.matmul(
                                out=ps_e[w * WIN:(w + 1) * WIN,
                                         i * 2 * H:(i + 1) * 2 * H],
                                lhsT=oneh_sb[:, gbcol * WIN:
                                             (gbcol + 1) * WIN],
                                rhs=rhs_ap,
                                start=st, stop=sp_,
                                tile_position=(0, w * WIN))
                    # agg = numer/(denom+1e-16) + z  (batched over group)
                    pe3 = ps_e[:, :ng * 2 * H].rearrange(
                        "p (t f) -> p t f", f=2 * H)
                    mlpin = grp_pool.tile([128, GROUP * H], bf16, tag="mlpin")
                    mi3 = mlpin[:, :ng * H].rearrange("p (t f) -> p t f", f=H)
                    rec = grp_pool.tile([128, GROUP * H], f32, tag="rec")
                    nc.vector.reciprocal(
                        out=rec[:, :ng * H].rearrange("p (t f) -> p t f", f=H),
                        in_=pe3[:, :, 0:H])
                    nc.vector.tensor_tensor(
                        out=mi3, in0=pe3[:, :, H:2 * H],
                        in1=rec[:, :ng * H].rearrange("p (t f) -> p t f", f=H),
                        op=OP.mult)
                    zsl = slice(tiles[0] * H, (tiles[-1] + 1) * H)
                    nc.vector.tensor_tensor(
                        out=mi3, in0=mi3,
                        in1=z_cur[:, zsl].rearrange("p (t f) -> p t f", f=H),
                        op=OP.add)

                    # --- MLP part 1: y1 = mlpin @ W1 (per tile) ---
                    ps_y1 = ppy.tile([128, GROUP * 2 * H], f32, tag="y1")
                    ps_t = ppt.tile([128, GROUP * 128], bf16, tag="tr")
                    for i, t in enumerate(tiles):
                        nc.tensor.transpose(
                            out=ps_t[:H, i * 128:(i + 1) * 128],
                            in_=mlpin[:, i * H:(i + 1) * H],
                            identity=ident_bf[:, :])
                    mT = wp.tile([128, GROUP * 128], bf16, tag="lhsb2")
                    nc.vector.tensor_copy(
                        out=mT[:H, :ng * 128], in_=ps_t[:H, :ng * 128])
                    for i, t in enumerate(tiles):
                        nc.tensor.matmul(
                            out=ps_y1[:, i * 2 * H:(i + 1) * 2 * H],
                            lhsT=mT[:H, i * 128:(i + 1) * 128],
                            rhs=w1_sb[:, li * 2 * H:(li + 1) * 2 * H],
                            start=True, stop=True)
                    # --- LN1 + relu (batched over group) ---
                    py3 = ps_y1[:, :ng * 2 * H].rearrange(
                        "p (t f) -> p t f", f=2 * H)
                    cent = grp_pool.tile([128, GROUP * 2 * H], f32, tag="cent")
                    c3 = cent[:, :ng * 2 * H].rearrange(
                        "p (t f) -> p t f", f=2 * H)
                    if not b1_triv:
                        nc.vector.tensor_tensor(
                            out=py3, in0=py3,
                            in1=freb(b1_sb[0:1, li * 2 * H:(li + 1) * 2 * H], ng),
                            op=OP.add)
                    s1m = wp.tile([128, GROUP], f32, tag="mu1")
                    nc.vector.reduce_sum(
                        out=s1m[:, :ng], in_=py3, axis=AX.X)
                    sq = grp_pool.tile([128, GROUP * 2 * H], bf16, tag="sq")
                    nc.scalar.activation(
                        out=sq[:, :ng * 2 * H], in_=ps_y1[:, :ng * 2 * H],
                        func=AF.Square)
                    s2m = wp.tile([128, GROUP], f32, tag="v1")
                    nc.vector.reduce_sum(
                        out=s2m[:, :ng],
                        in_=sq[:, :ng * 2 * H].rearrange(
                            "p (t f) -> p t f", f=2 * H),
                        axis=AX.X)
                    t1m = wp.tile([128, GROUP], f32, tag="t1m")
                    nc.vector.scalar_tensor_tensor(
                        out=t1m[:, :ng], in0=s1m[:, :ng],
                        scalar=1.0 / (4 * H * H),
                        in1=s1m[:, :ng], op0=OP.mult, op1=OP.mult)
                    nc.vector.scalar_tensor_tensor(
                        out=s2m[:, :ng], in0=s2m[:, :ng], scalar=1.0 / (2 * H),
                        in1=t1m[:, :ng], op0=OP.mult, op1=OP.subtract)
                    nc.scalar.activation(
                        out=s2m[:, :ng], in_=s2m[:, :ng], func=AF.Ln,
                        bias=b_ln, scale=1.0)
                    rs1 = wp.tile([128, GROUP], f32, tag="rs1")
                    nc.scalar.activation(
                        out=rs1[:, :ng], in_=s2m[:, :ng], func=AF.Exp,
                        scale=-0.5)
                    mu1 = wp.tile([128, GROUP], f32, tag="mu1b")
                    nc.vector.tensor_scalar(
                        out=mu1[:, :ng], in0=s1m[:, :ng],
                        scalar1=1.0 / (2 * H), scalar2=None, op0=OP.mult)
                    nc.vector.tensor_tensor(
                        out=c3, in0=py3,
                        in1=mu1[:, :ng].broadcast_to([128, ng, 2 * H]),
                        op=OP.subtract)
                    z2 = grp_pool.tile([128, GROUP * 2 * H], bf16, tag="z2")
                    z23 = z2[:, :ng * 2 * H].rearrange(
                        "p (t f) -> p t f", f=2 * H)
                    if ln1_triv:
                        for i in range(ng):
                            nc.scalar.activation(
                                out=z2[:, i * 2 * H:(i + 1) * 2 * H],
                                in_=cent[:, i * 2 * H:(i + 1) * 2 * H],
                                func=AF.Relu, scale=rs1[:, i:i + 1])
                    else:
                        nc.vector.tensor_tensor(
                            out=z23, in0=c3,
                            in1=rs1[:, :ng].broadcast_to([128, ng, 2 * H]),
                            op=OP.mult)
                        nc.vector.tensor_tensor(
                            out=z23, in0=z23,
                            in1=freb(ln1g_sb[0:1, li * 2 * H:(li + 1) * 2 * H],
                                     ng),
                            op=OP.mult)
                        nc.vector.tensor_tensor(
                            out=z23, in0=z23,
                            in1=freb(ln1b_sb[0:1, li * 2 * H:(li + 1) * 2 * H],
                                     ng),
                            op=OP.add)
                        nc.scalar.activation(
                            out=z2[:, :ng * 2 * H], in_=z2[:, :ng * 2 * H],
                            func=AF.Relu)
                    # --- MLP part 2: y2 = z2 @ W2 ; h update ---
                    ps_y2 = pp1.tile([128, GROUP * H], f32, tag="y2")
                    ps_t2 = ppb.tile([128, 2 * GROUP * 128], bf16, tag="trb")
                    for i, t in enumerate(tiles):
                        nc.tensor.transpose(
                            out=ps_t2[:, i * 128:(i + 1) * 128],
                            in_=z2[:, i * 2 * H:(i + 1) * 2 * H],
                            identity=ident_bf[:, :])
                    zT = wp.tile([128, GROUP * 128], bf16, tag="lhsb2")
                    nc.vector.tensor_copy(
                        out=zT[:, :ng * 128], in_=ps_t2[:, :ng * 128])
                    for i, t in enumerate(tiles):
                        nc.tensor.matmul(
                            out=ps_y2[:, i * H:(i + 1) * H],
                            lhsT=zT[:, i * 128:(i + 1) * 128],
                            rhs=w2_sb[:, li * H:(li + 1) * H],
                            start=True, stop=True)
                    py2_3 = ps_y2[:, :ng * H].rearrange(
                        "p (t f) -> p t f", f=H)
                    hsl = slice(tiles[0] * H, (tiles[-1] + 1) * H)
                    if not b2_triv:
                        nc.vector.tensor_tensor(
                            out=py2_3, in0=py2_3,
                            in1=freb(b2_sb[0:1, li * H:(li + 1) * H], ng),
                            op=OP.add)
                    if l == 0:
                        nc.vector.tensor_copy(
                            out=h_sb[:, hsl], in_=ps_y2[:, :ng * H])
                    else:
                        nc.vector.tensor_tensor(
                            out=h_sb[:, hsl], in0=ps_y2[:, :ng * H],
                            in1=h_sb[:, hsl], op=OP.add)
                    # overlap the next node phase / final head with the
                    # remaining groups' gather DMA
                    if l + 1 < nlayers:
                        node_phase(l + 1, tiles)
                        if g_i == 2:
                            publish_A(l + 1)
                    else:
                        if tiles[0] < 24:
                            final_phase(tiles)
                        elif tiles[0] == 24:
                            final_ln(tiles, 0)
                        else:
                            final_ln(tiles, 4)
                            final_head6()
                if l + 1 < nlayers:
                    publish_F(l + 1)

    nc.compile()
    return nc


# --------------------------------------------------------------------------
# Entry point
# --------------------------------------------------------------------------

def kernel(x, edge_index, enc_W, enc_b, t, W1, b1, ln1_g, ln1_b, W2, b2,
           norm_g, norm_b, lin_W, lin_b):
    global LAST_RESULTS
    from concourse.bass_utils import run_bass_kernel_spmd

    x = np.ascontiguousarray(np.asarray(x, dtype=np.float32))
    edge_index = np.asarray(edge_index)
    key = hash((edge_index.tobytes(),))

    triv = dict(
        t=bool(np.allclose(np.asarray(t), 1.0)),
        ln1=bool(np.allclose(np.asarray(ln1_g), 1.0)
                 and np.allclose(np.asarray(ln1_b), 0.0)),
        b1=bool(np.allclose(np.asarray(b1), 0.0)),
        b2=bool(np.allclose(np.asarray(b2), 0.0)),
        encb=bool(np.allclose(np.asarray(enc_b), 0.0)),
        linb=bool(np.allclose(np.asarray(lin_b), 0.0)),
        norm=bool(np.allclose(np.asarray(norm_g), 1.0)
                  and np.allclose(np.asarray(norm_b), 0.0)),
    )
    global _last_triv
    _last_triv = triv
    ckey = (key, tuple(sorted(triv.items())))
    if ckey in _CACHE:
        meta, nc = _CACHE[ckey]
    else:
        meta = _preprocess(edge_index)
        nc = _build(meta, triv)
        _CACHE.clear()
        _CACHE[ckey] = (meta, nc)

    f32c = lambda a: np.ascontiguousarray(np.asarray(a, dtype=np.float32))
    node_of = meta["node_of"]
    L2H = 2 * H

    shared = dict(
        encW=np.ascontiguousarray(np.asarray(enc_W, dtype=np.float32)
                                  .astype(ml_dtypes.bfloat16)),
        w1=np.ascontiguousarray(np.transpose(np.asarray(W1, dtype=np.float32),
                                   (1, 0, 2)).astype(ml_dtypes.bfloat16)),
        w2=np.ascontiguousarray(np.transpose(np.asarray(W2, dtype=np.float32),
                                   (1, 0, 2)).astype(ml_dtypes.bfloat16)),
        linW=np.ascontiguousarray(np.asarray(lin_W, dtype=np.float32)
                                  .astype(ml_dtypes.bfloat16)),
        smallc=np.concatenate([
            f32c(enc_b).reshape(-1), f32c(t).reshape(-1),
            f32c(norm_g).reshape(-1), f32c(norm_b).reshape(-1),
            f32c(ln1_g).reshape(-1), f32c(ln1_b).reshape(-1),
            f32c(b1).reshape(-1), f32c(b2).reshape(-1),
            f32c(lin_b).reshape(-1)]).reshape(1, 2412),
    )

    in_maps = []
    for c in range(NC_):
        xs = np.zeros((NPC, F_IN), np.float32)
        valid = node_of[c] >= 0
        xs[valid] = x[node_of[c][valid]]
        m = dict(shared)
        # row r = p*TILES + t -> [128, TILES*F_IN] with partition-major rows
        m["x_sh"] = np.ascontiguousarray(
            xs.astype(ml_dtypes.bfloat16).reshape(128, TILES * F_IN))
        m["idxs"] = np.ascontiguousarray(meta["idx_slab"][c])
        m["oneh"] = np.ascontiguousarray(meta["oneh"][c])
        in_maps.append(m)

    def _run():
        try:
            return run_bass_kernel_spmd(nc, in_maps, core_ids=list(range(NC_)))
        except ModuleNotFoundError:
            # BASS_TRACE set but the axon NTFF hook module is unavailable
            import os
            os.environ["BASS_NEVER_TRACE"] = "1"
            return run_bass_kernel_spmd(nc, in_maps, core_ids=list(range(NC_)))

    out = np.empty((N, C), np.float32)
    for attempt in range(3):
        res = _run()
        LAST_RESULTS = res
        for c in range(NC_):
            o = np.asarray(res.results[c]["out"]).reshape(NPC, C)
            valid = node_of[c] >= 0
            out[node_of[c][valid]] = o[valid]
        if np.isfinite(out).all():
            break
    return out
